# revision 9
# baseline (speedup 1.0000x reference)
"""Self-attention kernel for TRN2: out = softmax(X Wq (X Wk)^T / sqrt(D)) @ X.

Strategy (8-way sequence parallelism over query rows):
  scores = (X Wqs)(X Wk)^T = X A  with  A^T = Wk (Wqs^T X_i^T), Wqs = Wq/sqrt(D)
so K and M = Wqs Wk^T are never materialized. Each core i handles query rows
[i*B, (i+1)*B):
  phase 0 (two streamed GEMM steps, per B-half):
    step1: T1 = Wqs^T X_i^T  (3-pass f32r hi/lo: hh+hl+lh — T1 errors amplify
           by ~D into the logits, so crosses must stay near-exact)
    step2: A^T = Wk T1       (3-pass f32r; same sensitivity via sqrt(D)*|X|)
    outputs: A^T as f32r hi tiles (aith) + e4m3 cross operands
           h8a = hi(A)*2^-9, l8a = lo(A)*2^4 in DoubleRow pair layout.
  flash:   stream key blocks j; logits S^T_j = X_j A in key-major layout as
           f32r hh pass + TWO fp8 DoubleRow cross passes (0.5 cyc/row):
             term1 = lo(X)*2^9 (stationary) x hi(A)*2^-9 (moving)
             term2 = hi(X)*2^-4 x lo(A)*2^4
           running column-max via PE transpose + reduce; E = exp(S - max)
           f32r; second matmul E^T-slices @ X_aug in f32r; ones-column in
           X_aug accumulates the softmax denominator; fused
           rescale-accumulate (acc = acc*corr + psum) on DVE; final divide.

Numerics: logits need ~17-bit abs precision (std ~1024, near-tie rows
amplify errors through softmax). f32r hi/lo split leaves cross terms at
~2^-12 relative, so quantizing THOSE to e4m3 (3-bit mantissa) only adds
~7e-3 logit noise — well under the ~0.04 budget. The P@X matmul only needs
~1e-3 relative, so a single f32r pass is safe there.

Engine placement tuned so DVE (~27us/SB) stays under PE (~34us/SB):
f32r hi casts + fp8 hi casts + exp + er cast + xar cast on ScalarE,
lo subtract + fp8 lo cast split between DVE and GpSimd.
"""
import numpy as np
from contextlib import ExitStack

import concourse.bass as bass
import concourse.bacc as bacc
import concourse.tile as tile
from concourse import mybir
from concourse.bass_utils import run_bass_kernel_spmd
from concourse.masks import make_identity

P = 128
SEQ = 8192
DIM = 1024
NCORES = 8
AUG = 4      # extra columns on X_aug: [ones, 0, 0, 0]
SBN = 4      # key n-tiles (of 128) per flash super-block

F32 = mybir.dt.float32
F32R = mybir.dt.float32r
F8 = mybir.dt.float8e4
EXP = mybir.ActivationFunctionType.Exp
ALU = mybir.AluOpType
AXX = mybir.AxisListType.X
DR = mybir.MatmulPerfMode.DoubleRow

# fp8 cross-term scales (product of each pair = 1.0)
S_LX = 512.0      # lo(X) * 2^9   (stationary, term1)
S_HA = 1.0 / 512.0  # hi(A) * 2^-9  (moving, term1)
S_HX = 1.0 / 16.0   # hi(X) * 2^-4  (stationary, term2)
S_LA = 16.0         # lo(A) * 2^4   (moving, term2)


def _chunks(total, step=512):
    return [(lo, min(lo + step, total)) for lo in range(0, total, step)]


def build_core_kernel(S, D, B, sbn=SBN, aug=AUG):
    """One core's kernel: query rows block of size B, full S keys."""
    KT = D // P      # contraction tiles over D
    NT = S // P      # key tiles
    MT = B // P      # query tiles (per core)
    NSB = NT // sbn  # super-blocks
    NPAIR = KT // 2  # DoubleRow contraction pairs
    XAW = D + aug
    assert NT % sbn == 0 and B % P == 0 and D % P == 0 and MT <= P and KT % 2 == 0

    nc = bacc.Bacc("TRN2", target_bir_lowering=False, debug=False)
    xtj = nc.dram_tensor("xtj", [NT, P, D], F32, kind="ExternalInput")
    xa = nc.dram_tensor("xa", [S, XAW], F32, kind="ExternalInput")
    wqs = nc.dram_tensor("wqs", [D, D], F32, kind="ExternalInput")
    wkt = nc.dram_tensor("wkt", [D, D], F32, kind="ExternalInput")
    xit = nc.dram_tensor("xit", [D, B], F32, kind="ExternalInput")
    out = nc.dram_tensor("out", [B, D], F32, kind="ExternalOutput")

    def pair_st(t, u):
        # stationary fp8 pair view [P, 2, P] of a [P, D] tile, pair u
        return t[:, u * 2 * P:(u + 1) * 2 * P].rearrange("p (i m) -> p i m", i=2)

    with tile.TileContext(nc) as tc, ExitStack() as ctx:
        pers = ctx.enter_context(tc.tile_pool(name="pers", bufs=1))
        aith = [pers.tile([P, B], F32R, name=f"aith{k}") for k in range(KT)]
        h8a = pers.tile([P, KT, B], F8, name="h8a")
        l8a = pers.tile([P, KT, B], F8, name="l8a")
        gm = pers.tile([P, B], F32, name="gm")
        mxbc = pers.tile([P, B], F32, name="mxbc")
        ident = pers.tile([P, P], F32, name="ident")
        make_identity(nc, ident[:])
        nc.gpsimd.memset(gm[:], -1e30)

        # ---- phase 0: T1 = Wqs^T X_i^T ; A^T = Wk T1 (per B-half) ----
        with ExitStack() as p0:
            t1p = p0.enter_context(tc.tile_pool(name="t1p", bufs=1))
            wp = p0.enter_context(tc.tile_pool(name="wp", bufs=2))
            xip = p0.enter_context(tc.tile_pool(name="xip", bufs=1))
            auxp = p0.enter_context(tc.tile_pool(name="auxp", bufs=2))
            ps0 = p0.enter_context(tc.tile_pool(name="ps0", bufs=4, space="PSUM"))
            HB = 512
            hT1 = t1p.tile([P, KT, HB], F32R, name="hT1")
            lT1 = t1p.tile([P, KT, HB], F32R, name="lT1")
            for (lo, hi) in _chunks(B):
                # X_i^T half-split: hi/lo f32r
                xi_f = xip.tile([P, KT, HB], F32, name=f"xif{lo}", tag="xif")
                for g in range(KT):
                    nc.sync.dma_start(xi_f[:, g, :], xit.ap()[g * P:(g + 1) * P, lo:hi])
                hxi = xip.tile([P, KT, HB], F32R, name=f"hxi{lo}", tag="hxi")
                nc.scalar.copy(hxi[:], xi_f[:])
                nc.vector.tensor_sub(xi_f[:], xi_f[:], hxi[:].bitcast(F32))
                lxi = xip.tile([P, KT, HB], F32R, name=f"lxi{lo}", tag="lxi")
                nc.vector.tensor_copy(lxi[:], xi_f[:])

                # step1: T1[r-tile, half] = sum_g Wqs[g,:][:, r]^T X^T[g, half]
                for r in range(KT):
                    wq_f = wp.tile([P, KT, P], F32, name=f"wqf{lo}_{r}", tag="wqf")
                    for g in range(KT):
                        nc.sync.dma_start(wq_f[:, g, :], wqs.ap()[g * P:(g + 1) * P, r * P:(r + 1) * P])
                    hwq = wp.tile([P, KT, P], F32R, name=f"hwq{lo}_{r}", tag="hwq")
                    nc.scalar.copy(hwq[:], wq_f[:])
                    nc.vector.tensor_sub(wq_f[:], wq_f[:], hwq[:].bitcast(F32))
                    lwq = wp.tile([P, KT, P], F32R, name=f"lwq{lo}_{r}", tag="lwq")
                    nc.vector.tensor_copy(lwq[:], wq_f[:])
                    t1ps = ps0.tile([P, HB], F32, name=f"t1ps{lo}_{r}", tag="pm")
                    n = 3 * KT
                    i = 0
                    for g in range(KT):
                        for (la, rb) in ((hwq, hxi), (hwq, lxi), (lwq, hxi)):
                            nc.tensor.matmul(t1ps[:], la[:, g, :], rb[:, g, :],
                                             start=(i == 0), stop=(i == n - 1))
                            i += 1
                    nc.vector.tensor_copy(hT1[:, r, :], t1ps[:])
                    nc.vector.tensor_sub(t1ps[:], t1ps[:], hT1[:, r, :].bitcast(F32))
                    nc.vector.tensor_copy(lT1[:, r, :], t1ps[:])

                # step2: A^T[d-tile, half] = sum_r Wk[:, r][d, :] T1[r, half]
                # (reuses the step1 weight-split slots: same tags/shapes)
                for d in range(KT):
                    wk_f = wp.tile([P, KT, P], F32, name=f"wkf{lo}_{d}", tag="wqf")
                    for r in range(KT):
                        nc.sync.dma_start(wk_f[:, r, :], wkt.ap()[r * P:(r + 1) * P, d * P:(d + 1) * P])
                    hwk = wp.tile([P, KT, P], F32R, name=f"hwk{lo}_{d}", tag="hwq")
                    nc.scalar.copy(hwk[:], wk_f[:])
                    nc.vector.tensor_sub(wk_f[:], wk_f[:], hwk[:].bitcast(F32))
                    lwk = wp.tile([P, KT, P], F32R, name=f"lwk{lo}_{d}", tag="lwq")
                    nc.vector.tensor_copy(lwk[:], wk_f[:])
                    pa = ps0.tile([P, HB], F32, name=f"pa{lo}_{d}", tag="pm")
                    n = 3 * KT
                    i = 0
                    for r in range(KT):
                        for (la, rb) in ((hwk, hT1), (hwk, lT1), (lwk, hT1)):
                            nc.tensor.matmul(pa[:], la[:, r, :], rb[:, r, :],
                                             start=(i == 0), stop=(i == n - 1))
                            i += 1
                    nc.vector.tensor_copy(aith[d][:, lo:hi], pa[:])
                    nc.scalar.mul(h8a[:, d, lo:hi], aith[d][:, lo:hi].bitcast(F32), S_HA)
                    al_f = auxp.tile([P, HB], F32, name=f"alf{lo}_{d}", tag="alf")
                    nc.vector.tensor_sub(al_f[:], pa[:], aith[d][:, lo:hi].bitcast(F32))
                    nc.vector.tensor_scalar_mul(l8a[:, d, lo:hi], al_f[:], S_LA)

        # ---- flash over key super-blocks ----
        accp = ctx.enter_context(tc.tile_pool(name="accp", bufs=1))
        acc = [accp.tile([P, XAW], F32, name=f"acc{t}") for t in range(MT)]
        for t in range(MT):
            nc.gpsimd.memset(acc[t][:], 0.0)
        sp = ctx.enter_context(tc.tile_pool(name="sp", bufs=2 * sbn + 2))
        erp = sp
        xap = ctx.enter_context(tc.tile_pool(name="xap", bufs=3))
        xarp = ctx.enter_context(tc.tile_pool(name="xarp", bufs=sbn + 1))
        xtp = xap
        xthp = ctx.enter_context(tc.tile_pool(name="xthp", bufs=3))
        stat = ctx.enter_context(tc.tile_pool(name="stat", bufs=2))
        ps_s = ctx.enter_context(tc.tile_pool(name="ps_s", bufs=2, space="PSUM"))
        ps_o = ctx.enter_context(tc.tile_pool(name="ps_o", bufs=2, space="PSUM"))
        ps_t = ps_s

        # Manual logical-time slots, one-super-block lag: the XT split-casts
        # for block s run during block s-1's PE burst (prep slot), the
        # stats/exp chain for block s runs during block s+1's burst, and the
        # f32r out-matmuls for block s run as a clean burst after block s+1's
        # S-matmuls. PE never waits on the DVE chain, and same-dtype matmuls
        # stay contiguous (mode switches serialize LDWEIGHTS).
        omx = None
        for s in range(NSB):
            js = list(range(s * sbn, (s + 1) * sbn))
            ssb, xar = [], []
            xsplit = []
            if True:  # prep (emission order only; no sim-time slots)
                for j in js:
                    xt_t = xtp.tile([P, XAW], F32, name=f"xt{j}", tag="stg")
                    nc.sync.dma_start(xt_t[:, :D], xtj.ap()[j])
                    xth = xthp.tile([P, D], F32R, name=f"xth{j}", tag="xth")
                    nc.scalar.copy(xth[:], xt_t[:, :D])
                    nc.vector.tensor_sub(xt_t[:, :D], xt_t[:, :D], xth[:].bitcast(F32))
                    l8x = xthp.tile([P, D], F8, name=f"l8x{j}", tag="l8x")
                    nc.gpsimd.tensor_scalar_mul(l8x[:], xt_t[:, :D], S_LX)
                    h8x = xthp.tile([P, D], F8, name=f"h8x{j}", tag="h8x")
                    nc.scalar.mul(h8x[:], xth[:].bitcast(F32), S_HX)
                    xsplit.append((xth, l8x, h8x))

            if True:
                for idx, j in enumerate(js):
                    xth, l8x, h8x = xsplit[idx]
                    s_t = sp.tile([P, B], F32, name=f"s{j}", tag="s")
                    pss = [ps_s.tile([P, 512], F32, name=f"pss{j}_{c}", tag="pss")
                           for c in range(2)]
                    # f32r hh pass, both chunks back-to-back (same PE mode)
                    for c, (lo, hi) in enumerate(_chunks(B)):
                        for k in range(KT):
                            nc.tensor.matmul(pss[c][:], xth[:, k * P:(k + 1) * P],
                                             aith[k][:, lo:hi], start=(k == 0), stop=(k == KT - 1))
                    # fp8 DoubleRow cross passes: each accumulates onto the
                    # closed f32r group via has_written (start=False); stop is
                    # sim-only bookkeeping so every DR matmul closes itself
                    for c, (lo, hi) in enumerate(_chunks(B)):
                        for u in range(NPAIR):
                            nc.tensor.matmul(pss[c][:], pair_st(l8x, u),
                                             h8a[:, 2 * u:2 * u + 2, lo:hi],
                                             start=False, stop=True, perf_mode=DR,
                                             skip_group_check=True)
                        for u in range(NPAIR):
                            nc.tensor.matmul(pss[c][:], pair_st(h8x, u),
                                             l8a[:, 2 * u:2 * u + 2, lo:hi],
                                             start=False, stop=True, perf_mode=DR,
                                             skip_group_check=True)
                    for c, (lo, hi) in enumerate(_chunks(B)):
                        nc.scalar.copy(s_t[:, lo:hi], pss[c][:])
                        nc.vector.tensor_max(gm[:, lo:hi], gm[:, lo:hi], pss[c][:])
                    ssb.append(s_t)

            if True:
                # per-query-column running max (transpose-reduce gm chunks)
                nmx = stat.tile([P, MT], F32, name=f"nmx{s}", tag="nmx")
                corr = stat.tile([P, MT], F32, name=f"corr{s}", tag="corr")
                for c in range(MT):
                    pt = ps_t.tile([P, P], F32, name=f"pt{s}_{c}", tag="pss")
                    nc.tensor.transpose(pt[:], gm[:, c * P:(c + 1) * P], ident[:])
                    nc.vector.reduce_max(nmx[:, c:c + 1], pt[:], axis=AXX)
                if omx is None:
                    nc.vector.memset(corr[:], 0.0)
                else:
                    dmx = stat.tile([P, MT], F32, name=f"dmx{s}", tag="dmx")
                    nc.vector.tensor_sub(dmx[:], omx[:], nmx[:])
                    nc.scalar.activation(corr[:], dmx[:], EXP)
                omx = nmx

                # broadcast nmx (query-major) -> mxbc [P, B] (key-major free)
                ptb = ps_t.tile([P, P], F32, name=f"ptb{s}", tag="pss")
                nc.tensor.transpose(ptb[:MT, :], nmx[:], ident[:])
                mtmp = stat.tile([MT, P], F32, name=f"mtmp{s}", tag="mtmp")
                nc.scalar.copy(mtmp[:], ptb[:MT, :])
                mrow = stat.tile([1, B], F32, name=f"mrow{s}", tag="mrow", bufs=1)
                nc.sync.dma_start(mrow[:].rearrange("a (b c) -> a b c", b=MT), mtmp[:])
                nc.gpsimd.partition_broadcast(mxbc[:], mrow[:])

            if True:
                # E = exp(S - max) in fp32 in place, then cast to f32r on
                # ScalarE (the cast must be the f32r memory's only writer)
                ers = []
                for idx, s_t in enumerate(ssb):
                    nc.vector.tensor_sub(s_t[:], s_t[:], mxbc[:])
                    nc.scalar.activation(s_t[:], s_t[:], EXP)
                    er_t = erp.tile([P, B], F32R, name=f"er{s}_{idx}", tag="s")
                    nc.scalar.copy(er_t[:], s_t[:])
                    ers.append(er_t)

            if True:
                for idx, j in enumerate(js):
                    xa_t = xap.tile([P, XAW], F32, name=f"xa{j}", tag="stg")
                    nc.sync.dma_start(xa_t[:], xa.ap()[j * P:(j + 1) * P, :])
                    xar_t = xarp.tile([P, XAW], F32R, name=f"xar{j}", tag="xar")
                    nc.scalar.copy(xar_t[:], xa_t[:])
                    xar.append(xar_t)

            if True:
                # out accumulation: acc = acc*corr + E^T @ X_aug (f32r burst);
                # 3 matmul streams into bank-aligned slices of one PSUM tile,
                # then a single fused rescale-accumulate per query tile
                for t in range(MT):
                    po = ps_o.tile([P, XAW], F32, name=f"po{s}_{t}", tag="po")
                    # idx outer so the 3 column chunks reuse one stationary
                    # operand back-to-back (LDWEIGHTS locality); each chunk's
                    # PSUM accumulation group still spans idx 0..sbn-1
                    for idx in range(sbn):
                        er = ers[idx][:]
                        for (lo, hi) in _chunks(XAW):
                            nc.tensor.matmul(po[:, lo:hi], er[:, t * P:(t + 1) * P],
                                             xar[idx][:, lo:hi], start=(idx == 0), stop=(idx == sbn - 1))
                    nc.vector.scalar_tensor_tensor(acc[t][:], acc[t][:],
                                                   corr[:, t:t + 1], po[:],
                                                   op0=ALU.mult, op1=ALU.add)

        # ---- finalize: divide by the ones-column sums, write out ----
        if True:
            for t in range(MT):
                rc = stat.tile([P, 1], F32, name=f"rc{t}", tag="rc")
                nc.vector.reciprocal(rc[:], acc[t][:, D:D + 1])
                nc.vector.tensor_scalar_mul(acc[t][:, 0:D], acc[t][:, 0:D], rc[:])
                nc.sync.dma_start(out.ap()[t * P:(t + 1) * P, :], acc[t][:, 0:D])

    nc.compile()
    return nc


def prep_inputs(X, Wq, Wk, S, D, n_cores, aug=AUG):
    B = S // n_cores
    NT = S // P
    KT = D // P
    X = np.ascontiguousarray(X, np.float32)
    scale = np.float32(1.0 / np.sqrt(D))
    xtj = np.ascontiguousarray(
        X.reshape(NT, P, KT, P).transpose(0, 3, 2, 1).reshape(NT, P, D))
    xa = np.zeros((S, D + aug), np.float32)
    xa[:, :D] = X
    xa[:, D] = 1.0
    wqs = np.ascontiguousarray(np.asarray(Wq, np.float32) * scale)
    wkt = np.ascontiguousarray(np.asarray(Wk, np.float32).T)
    xt = X.T
    in_maps = []
    for i in range(n_cores):
        in_maps.append({
            "xtj": xtj, "xa": xa, "wqs": wqs, "wkt": wkt,
            "xit": np.ascontiguousarray(xt[:, i * B:(i + 1) * B]),
        })
    return in_maps


_CACHE = {}


def _get_kernel(S, D, B, sbn):
    key = (S, D, B, sbn)
    if key not in _CACHE:
        _CACHE[key] = build_core_kernel(S, D, B, sbn=sbn)
    return _CACHE[key]


def kernel(inputs, weight_query, weight_key):
    S, D = inputs.shape
    assert (S, D) == (SEQ, DIM)
    B = S // NCORES
    nc = _get_kernel(S, D, B, SBN)
    in_maps = prep_inputs(inputs, weight_query, weight_key, S, D, NCORES)
    res = run_bass_kernel_spmd(nc, in_maps, core_ids=list(range(NCORES)))
    return np.concatenate([res.results[i]["out"] for i in range(NCORES)], axis=0)


if __name__ == "__main__":
    rng = np.random.default_rng(0)
    X = rng.standard_normal((SEQ, DIM), dtype=np.float32)
    Wq = rng.standard_normal((DIM, DIM), dtype=np.float32)
    Wk = rng.standard_normal((DIM, DIM), dtype=np.float32)
    out = kernel(X, Wq, Wk)
    print(out.shape, out.dtype)


# revision 10
# speedup vs baseline: 1.4881x; 1.4881x over previous
"""Self-attention kernel for TRN2: out = softmax(X Wq (X Wk)^T / sqrt(D)) @ X.

Strategy (8-way sequence parallelism over query rows):
  scores = (X Wqs)(X Wk)^T = X A  with  A^T = Wk (Wqs^T X_i^T), Wqs = Wq/sqrt(D)
so K and M = Wqs Wk^T are never materialized. Each core i handles query rows
[i*B, (i+1)*B):
  phase 0 (two streamed GEMM steps, per B-half):
    step1: T1 = Wqs^T X_i^T  (3-pass f32r hi/lo: hh+hl+lh — T1 errors amplify
           by ~D into the logits, so crosses must stay near-exact)
    step2: A^T = Wk T1       (3-pass f32r; same sensitivity via sqrt(D)*|X|)
    outputs: A^T as f32r hi tiles (aith) + e4m3 cross operands
           h8a = hi(A)*2^-9, l8a = lo(A)*2^4 in DoubleRow pair layout.
  flash:   stream key blocks j; logits S^T_j = X_j A in key-major layout as
           f32r hh pass + TWO fp8 DoubleRow cross passes (0.5 cyc/row):
             term1 = lo(X)*2^9 (stationary) x hi(A)*2^-9 (moving)
             term2 = hi(X)*2^-4 x lo(A)*2^4
           running column-max via PE transpose + reduce; E = exp(S - max)
           f32r; second matmul E^T-slices @ X_aug in f32r; ones-column in
           X_aug accumulates the softmax denominator; fused
           rescale-accumulate (acc = acc*corr + psum) on DVE; final divide.

Numerics: logits need ~17-bit abs precision (std ~1024, near-tie rows
amplify errors through softmax). f32r hi/lo split leaves cross terms at
~2^-12 relative, so quantizing THOSE to e4m3 (3-bit mantissa) only adds
~7e-3 logit noise — well under the ~0.04 budget. The P@X matmul only needs
~1e-3 relative, so a single f32r pass is safe there.

Engine placement tuned so DVE (~27us/SB) stays under PE (~34us/SB):
f32r hi casts + fp8 hi casts + exp + er cast + xar cast on ScalarE,
lo subtract + fp8 lo cast split between DVE and GpSimd.
"""
import numpy as np
from contextlib import ExitStack

import concourse.bass as bass
import concourse.bacc as bacc
import concourse.tile as tile
from concourse import mybir
from concourse.bass_utils import run_bass_kernel_spmd
from concourse.masks import make_identity

P = 128
SEQ = 8192
DIM = 1024
NCORES = 8
AUG = 4      # extra columns on X_aug: [ones, 0, 0, 0]
SBN = 4      # key n-tiles (of 128) per flash super-block

F32 = mybir.dt.float32
F32R = mybir.dt.float32r
F8 = mybir.dt.float8e4
EXP = mybir.ActivationFunctionType.Exp
ALU = mybir.AluOpType
AXX = mybir.AxisListType.X
DR = mybir.MatmulPerfMode.DoubleRow

# fp8 cross-term scales (product of each pair = 1.0)
S_LX = 512.0      # lo(X) * 2^9   (stationary, term1)
S_HA = 1.0 / 512.0  # hi(A) * 2^-9  (moving, term1)
S_HX = 1.0 / 16.0   # hi(X) * 2^-4  (stationary, term2)
S_LA = 16.0         # lo(A) * 2^4   (moving, term2)


def _chunks(total, step=512):
    return [(lo, min(lo + step, total)) for lo in range(0, total, step)]


def build_core_kernel(S, D, B, sbn=SBN, aug=AUG):
    """One core's kernel: query rows block of size B, full S keys."""
    KT = D // P      # contraction tiles over D
    NT = S // P      # key tiles
    MT = B // P      # query tiles (per core)
    NSB = NT // sbn  # super-blocks
    NPAIR = KT // 2  # DoubleRow contraction pairs
    XAW = D + aug
    assert NT % sbn == 0 and B % P == 0 and D % P == 0 and MT <= P and KT % 2 == 0

    nc = bacc.Bacc("TRN2", target_bir_lowering=False, debug=False)
    xtj = nc.dram_tensor("xtj", [NT, P, D], F32, kind="ExternalInput")
    xa = nc.dram_tensor("xa", [S, XAW], F32, kind="ExternalInput")
    wqs = nc.dram_tensor("wqs", [D, D], F32, kind="ExternalInput")
    wkt = nc.dram_tensor("wkt", [D, D], F32, kind="ExternalInput")
    xit = nc.dram_tensor("xit", [D, B], F32, kind="ExternalInput")
    out = nc.dram_tensor("out", [B, D], F32, kind="ExternalOutput")

    def pair_st(t, u):
        # stationary fp8 pair view [P, 2, P] of a [P, D] tile, pair u
        return t[:, u * 2 * P:(u + 1) * 2 * P].rearrange("p (i m) -> p i m", i=2)

    with tile.TileContext(nc) as tc, ExitStack() as ctx:
        pers = ctx.enter_context(tc.tile_pool(name="pers", bufs=1))
        aith = [pers.tile([P, B], F32R, name=f"aith{k}") for k in range(KT)]
        h8a = pers.tile([P, KT, B], F8, name="h8a")
        l8a = pers.tile([P, KT, B], F8, name="l8a")
        gm = pers.tile([P, B], F32, name="gm")
        mxbc = pers.tile([P, B], F32, name="mxbc")
        ident = pers.tile([P, P], F32, name="ident")
        make_identity(nc, ident[:])
        nc.gpsimd.memset(gm[:], -1e30)

        # ---- phase 0: T1 = Wqs^T X_i^T ; A^T = Wk T1 (per B-half) ----
        with ExitStack() as p0:
            t1p = p0.enter_context(tc.tile_pool(name="t1p", bufs=1))
            wp = p0.enter_context(tc.tile_pool(name="wp", bufs=2))
            xip = p0.enter_context(tc.tile_pool(name="xip", bufs=1))
            auxp = p0.enter_context(tc.tile_pool(name="auxp", bufs=2))
            ps0 = p0.enter_context(tc.tile_pool(name="ps0", bufs=4, space="PSUM"))
            HB = 512
            hT1 = t1p.tile([P, KT, HB], F32R, name="hT1")
            lT1 = t1p.tile([P, KT, HB], F32R, name="lT1")
            for (lo, hi) in _chunks(B):
                # X_i^T half-split: hi/lo f32r
                xi_f = xip.tile([P, KT, HB], F32, name=f"xif{lo}", tag="xif")
                for g in range(KT):
                    nc.sync.dma_start(xi_f[:, g, :], xit.ap()[g * P:(g + 1) * P, lo:hi])
                hxi = xip.tile([P, KT, HB], F32R, name=f"hxi{lo}", tag="hxi")
                nc.scalar.copy(hxi[:], xi_f[:])
                nc.vector.tensor_sub(xi_f[:], xi_f[:], hxi[:].bitcast(F32))
                lxi = xip.tile([P, KT, HB], F32R, name=f"lxi{lo}", tag="lxi")
                nc.vector.tensor_copy(lxi[:], xi_f[:])

                # step1: T1[r-tile, half] = sum_g Wqs[g,:][:, r]^T X^T[g, half]
                for r in range(KT):
                    wq_f = wp.tile([P, KT, P], F32, name=f"wqf{lo}_{r}", tag="wqf")
                    for g in range(KT):
                        nc.sync.dma_start(wq_f[:, g, :], wqs.ap()[g * P:(g + 1) * P, r * P:(r + 1) * P])
                    hwq = wp.tile([P, KT, P], F32R, name=f"hwq{lo}_{r}", tag="hwq")
                    nc.scalar.copy(hwq[:], wq_f[:])
                    nc.vector.tensor_sub(wq_f[:], wq_f[:], hwq[:].bitcast(F32))
                    lwq = wp.tile([P, KT, P], F32R, name=f"lwq{lo}_{r}", tag="lwq")
                    nc.vector.tensor_copy(lwq[:], wq_f[:])
                    t1ps = ps0.tile([P, HB], F32, name=f"t1ps{lo}_{r}", tag="pm")
                    n = 3 * KT
                    i = 0
                    for g in range(KT):
                        for (la, rb) in ((hwq, hxi), (hwq, lxi), (lwq, hxi)):
                            nc.tensor.matmul(t1ps[:], la[:, g, :], rb[:, g, :],
                                             start=(i == 0), stop=(i == n - 1))
                            i += 1
                    nc.vector.tensor_copy(hT1[:, r, :], t1ps[:])
                    nc.vector.tensor_sub(t1ps[:], t1ps[:], hT1[:, r, :].bitcast(F32))
                    nc.vector.tensor_copy(lT1[:, r, :], t1ps[:])

                # step2: A^T[d-tile, half] = sum_r Wk[:, r][d, :] T1[r, half]
                # (reuses the step1 weight-split slots: same tags/shapes)
                for d in range(KT):
                    wk_f = wp.tile([P, KT, P], F32, name=f"wkf{lo}_{d}", tag="wqf")
                    for r in range(KT):
                        nc.sync.dma_start(wk_f[:, r, :], wkt.ap()[r * P:(r + 1) * P, d * P:(d + 1) * P])
                    hwk = wp.tile([P, KT, P], F32R, name=f"hwk{lo}_{d}", tag="hwq")
                    nc.scalar.copy(hwk[:], wk_f[:])
                    nc.vector.tensor_sub(wk_f[:], wk_f[:], hwk[:].bitcast(F32))
                    lwk = wp.tile([P, KT, P], F32R, name=f"lwk{lo}_{d}", tag="lwq")
                    nc.vector.tensor_copy(lwk[:], wk_f[:])
                    pa = ps0.tile([P, HB], F32, name=f"pa{lo}_{d}", tag="pm")
                    n = 3 * KT
                    i = 0
                    for r in range(KT):
                        for (la, rb) in ((hwk, hT1), (hwk, lT1), (lwk, hT1)):
                            nc.tensor.matmul(pa[:], la[:, r, :], rb[:, r, :],
                                             start=(i == 0), stop=(i == n - 1))
                            i += 1
                    nc.vector.tensor_copy(aith[d][:, lo:hi], pa[:])
                    nc.scalar.mul(h8a[:, d, lo:hi], aith[d][:, lo:hi].bitcast(F32), S_HA)
                    al_f = auxp.tile([P, HB], F32, name=f"alf{lo}_{d}", tag="alf")
                    nc.vector.tensor_sub(al_f[:], pa[:], aith[d][:, lo:hi].bitcast(F32))
                    nc.vector.tensor_scalar_mul(l8a[:, d, lo:hi], al_f[:], S_LA)

        # ---- flash over key super-blocks ----
        accp = ctx.enter_context(tc.tile_pool(name="accp", bufs=1))
        acc = [accp.tile([P, XAW], F32, name=f"acc{t}") for t in range(MT)]
        for t in range(MT):
            nc.gpsimd.memset(acc[t][:], 0.0)
        sp = ctx.enter_context(tc.tile_pool(name="sp", bufs=2 * sbn + 2))
        erp = sp
        xap = ctx.enter_context(tc.tile_pool(name="xap", bufs=3))
        xarp = ctx.enter_context(tc.tile_pool(name="xarp", bufs=sbn + 1))
        xtp = xap
        xthp = ctx.enter_context(tc.tile_pool(name="xthp", bufs=3))
        stat = ctx.enter_context(tc.tile_pool(name="stat", bufs=2))
        ps_s = ctx.enter_context(tc.tile_pool(name="ps_s", bufs=2, space="PSUM"))
        ps_o = ctx.enter_context(tc.tile_pool(name="ps_o", bufs=2, space="PSUM"))
        ps_t = ps_s

        # Manual logical-time slots, one-super-block lag: the XT split-casts
        # for block s run during block s-1's PE burst (prep slot), the
        # stats/exp chain for block s runs during block s+1's burst, and the
        # f32r out-matmuls for block s run as a clean burst after block s+1's
        # S-matmuls. PE never waits on the DVE chain, and same-dtype matmuls
        # stay contiguous (mode switches serialize LDWEIGHTS).
        omx = None
        for s in range(NSB):
            js = list(range(s * sbn, (s + 1) * sbn))
            ssb, xar = [], []
            xsplit = []
            if True:  # prep (emission order only; no sim-time slots)
                for j in js:
                    xt_t = xtp.tile([P, XAW], F32, name=f"xt{j}", tag="stg")
                    nc.sync.dma_start(xt_t[:, :D], xtj.ap()[j])
                    xth = xthp.tile([P, D], F32R, name=f"xth{j}", tag="xth")
                    nc.scalar.copy(xth[:], xt_t[:, :D])
                    nc.vector.tensor_sub(xt_t[:, :D], xt_t[:, :D], xth[:].bitcast(F32))
                    l8x = xthp.tile([P, D], F8, name=f"l8x{j}", tag="l8x")
                    nc.vector.tensor_scalar_mul(l8x[:], xt_t[:, :D], S_LX)
                    h8x = xthp.tile([P, D], F8, name=f"h8x{j}", tag="h8x")
                    nc.scalar.mul(h8x[:], xth[:].bitcast(F32), S_HX)
                    xsplit.append((xth, l8x, h8x))

            if True:
                for idx, j in enumerate(js):
                    xth, l8x, h8x = xsplit[idx]
                    s_t = sp.tile([P, B], F32, name=f"s{j}", tag="s")
                    pss = [ps_s.tile([P, 512], F32, name=f"pss{j}_{c}", tag="pss")
                           for c in range(2)]
                    # f32r hh pass, both chunks back-to-back (same PE mode)
                    for c, (lo, hi) in enumerate(_chunks(B)):
                        for k in range(KT):
                            nc.tensor.matmul(pss[c][:], xth[:, k * P:(k + 1) * P],
                                             aith[k][:, lo:hi], start=(k == 0), stop=(k == KT - 1))
                    # fp8 DoubleRow cross passes: each accumulates onto the
                    # closed f32r group via has_written (start=False); stop is
                    # sim-only bookkeeping so every DR matmul closes itself
                    for c, (lo, hi) in enumerate(_chunks(B)):
                        for u in range(NPAIR):
                            nc.tensor.matmul(pss[c][:], pair_st(l8x, u),
                                             h8a[:, 2 * u:2 * u + 2, lo:hi],
                                             start=False, stop=True, perf_mode=DR,
                                             skip_group_check=True)
                        for u in range(NPAIR):
                            nc.tensor.matmul(pss[c][:], pair_st(h8x, u),
                                             l8a[:, 2 * u:2 * u + 2, lo:hi],
                                             start=False, stop=True, perf_mode=DR,
                                             skip_group_check=True)
                    for c, (lo, hi) in enumerate(_chunks(B)):
                        nc.scalar.copy(s_t[:, lo:hi], pss[c][:])
                        nc.vector.tensor_max(gm[:, lo:hi], gm[:, lo:hi], pss[c][:])
                    ssb.append(s_t)

            if True:
                # per-query-column running max (transpose-reduce gm chunks)
                nmx = stat.tile([P, MT], F32, name=f"nmx{s}", tag="nmx")
                corr = stat.tile([P, MT], F32, name=f"corr{s}", tag="corr")
                for c in range(MT):
                    pt = ps_t.tile([P, P], F32, name=f"pt{s}_{c}", tag="pss")
                    nc.tensor.transpose(pt[:], gm[:, c * P:(c + 1) * P], ident[:])
                    nc.vector.reduce_max(nmx[:, c:c + 1], pt[:], axis=AXX)
                if omx is None:
                    nc.vector.memset(corr[:], 0.0)
                else:
                    dmx = stat.tile([P, MT], F32, name=f"dmx{s}", tag="dmx")
                    nc.vector.tensor_sub(dmx[:], omx[:], nmx[:])
                    nc.scalar.activation(corr[:], dmx[:], EXP)
                omx = nmx

                # broadcast nmx (query-major) -> mxbc [P, B] (key-major free)
                ptb = ps_t.tile([P, P], F32, name=f"ptb{s}", tag="pss")
                nc.tensor.transpose(ptb[:MT, :], nmx[:], ident[:])
                mtmp = stat.tile([MT, P], F32, name=f"mtmp{s}", tag="mtmp")
                nc.scalar.copy(mtmp[:], ptb[:MT, :])
                mrow = stat.tile([1, B], F32, name=f"mrow{s}", tag="mrow", bufs=1)
                nc.sync.dma_start(mrow[:].rearrange("a (b c) -> a b c", b=MT), mtmp[:])
                nc.gpsimd.partition_broadcast(mxbc[:], mrow[:])

            if True:
                # E = exp(S - max) in fp32 in place, then cast to f32r on
                # ScalarE (the cast must be the f32r memory's only writer)
                ers = []
                for idx, s_t in enumerate(ssb):
                    nc.vector.tensor_sub(s_t[:], s_t[:], mxbc[:])
                    nc.scalar.activation(s_t[:], s_t[:], EXP)
                    er_t = erp.tile([P, B], F32R, name=f"er{s}_{idx}", tag="s")
                    nc.scalar.copy(er_t[:], s_t[:])
                    ers.append(er_t)

            if True:
                for idx, j in enumerate(js):
                    xa_t = xap.tile([P, XAW], F32, name=f"xa{j}", tag="stg")
                    nc.sync.dma_start(xa_t[:], xa.ap()[j * P:(j + 1) * P, :])
                    xar_t = xarp.tile([P, XAW], F32R, name=f"xar{j}", tag="xar")
                    nc.scalar.copy(xar_t[:], xa_t[:])
                    xar.append(xar_t)

            if True:
                # out accumulation: acc = acc*corr + E^T @ X_aug (f32r burst);
                # 3 matmul streams into bank-aligned slices of one PSUM tile,
                # then a single fused rescale-accumulate per query tile
                for t in range(MT):
                    po = ps_o.tile([P, XAW], F32, name=f"po{s}_{t}", tag="po")
                    # idx outer so the 3 column chunks reuse one stationary
                    # operand back-to-back (LDWEIGHTS locality); each chunk's
                    # PSUM accumulation group still spans idx 0..sbn-1
                    for idx in range(sbn):
                        er = ers[idx][:]
                        for (lo, hi) in _chunks(XAW):
                            nc.tensor.matmul(po[:, lo:hi], er[:, t * P:(t + 1) * P],
                                             xar[idx][:, lo:hi], start=(idx == 0), stop=(idx == sbn - 1))
                    nc.vector.scalar_tensor_tensor(acc[t][:], acc[t][:],
                                                   corr[:, t:t + 1], po[:],
                                                   op0=ALU.mult, op1=ALU.add)

        # ---- finalize: divide by the ones-column sums, write out ----
        if True:
            for t in range(MT):
                rc = stat.tile([P, 1], F32, name=f"rc{t}", tag="rc")
                nc.vector.reciprocal(rc[:], acc[t][:, D:D + 1])
                nc.vector.tensor_scalar_mul(acc[t][:, 0:D], acc[t][:, 0:D], rc[:])
                nc.sync.dma_start(out.ap()[t * P:(t + 1) * P, :], acc[t][:, 0:D])

    nc.compile()
    return nc


def prep_inputs(X, Wq, Wk, S, D, n_cores, aug=AUG):
    B = S // n_cores
    NT = S // P
    KT = D // P
    X = np.ascontiguousarray(X, np.float32)
    scale = np.float32(1.0 / np.sqrt(D))
    xtj = np.ascontiguousarray(
        X.reshape(NT, P, KT, P).transpose(0, 3, 2, 1).reshape(NT, P, D))
    xa = np.zeros((S, D + aug), np.float32)
    xa[:, :D] = X
    xa[:, D] = 1.0
    wqs = np.ascontiguousarray(np.asarray(Wq, np.float32) * scale)
    wkt = np.ascontiguousarray(np.asarray(Wk, np.float32).T)
    xt = X.T
    in_maps = []
    for i in range(n_cores):
        in_maps.append({
            "xtj": xtj, "xa": xa, "wqs": wqs, "wkt": wkt,
            "xit": np.ascontiguousarray(xt[:, i * B:(i + 1) * B]),
        })
    return in_maps


_CACHE = {}


def _get_kernel(S, D, B, sbn):
    key = (S, D, B, sbn)
    if key not in _CACHE:
        _CACHE[key] = build_core_kernel(S, D, B, sbn=sbn)
    return _CACHE[key]


def kernel(inputs, weight_query, weight_key):
    S, D = inputs.shape
    assert (S, D) == (SEQ, DIM)
    B = S // NCORES
    nc = _get_kernel(S, D, B, SBN)
    in_maps = prep_inputs(inputs, weight_query, weight_key, S, D, NCORES)
    res = run_bass_kernel_spmd(nc, in_maps, core_ids=list(range(NCORES)))
    return np.concatenate([res.results[i]["out"] for i in range(NCORES)], axis=0)


if __name__ == "__main__":
    rng = np.random.default_rng(0)
    X = rng.standard_normal((SEQ, DIM), dtype=np.float32)
    Wq = rng.standard_normal((DIM, DIM), dtype=np.float32)
    Wk = rng.standard_normal((DIM, DIM), dtype=np.float32)
    out = kernel(X, Wq, Wk)
    print(out.shape, out.dtype)


# revision 18
# speedup vs baseline: 1.6676x; 1.1206x over previous
"""Self-attention kernel for TRN2: out = softmax(X Wq (X Wk)^T / sqrt(D)) @ X.

Strategy (8-way sequence parallelism over query rows):
  scores = (X Wqs)(X Wk)^T = X A  with  A^T = Wk (Wqs^T X_i^T), Wqs = Wq/sqrt(D)
so K and M = Wqs Wk^T are never materialized. Each core i handles query rows
[i*B, (i+1)*B):
  phase 0 (two streamed GEMM steps, per B-half):
    step1: T1 = Wqs^T X_i^T  (3-pass f32r hi/lo: hh+hl+lh — T1 errors amplify
           by ~D into the logits, so crosses must stay near-exact)
    step2: A^T = Wk T1       (3-pass f32r; same sensitivity via sqrt(D)*|X|)
    outputs: A^T as f32r hi tiles (aith) + e4m3 cross operands
           h8a = hi(A)*2^-9, l8a = lo(A)*2^4 in DoubleRow pair layout.
  flash:   stream key blocks j; logits S^T_j = X_j A in key-major layout as
           f32r hh pass + TWO fp8 DoubleRow cross passes (0.5 cyc/row):
             term1 = lo(X)*2^9 (stationary) x hi(A)*2^-9 (moving)
             term2 = hi(X)*2^-4 x lo(A)*2^4
           running column-max via PE transpose + reduce; E = exp(S - max)
           f32r; second matmul E^T-slices @ X_aug in f32r; ones-column in
           X_aug accumulates the softmax denominator; fused
           rescale-accumulate (acc = acc*corr + psum) on DVE; final divide.

Numerics: logits need ~17-bit abs precision (std ~1024, near-tie rows
amplify errors through softmax). f32r hi/lo split leaves cross terms at
~2^-12 relative, so quantizing THOSE to e4m3 (3-bit mantissa) only adds
~7e-3 logit noise — well under the ~0.04 budget. The P@X matmul only needs
~1e-3 relative, so a single f32r pass is safe there.

Engine placement tuned so DVE (~27us/SB) stays under PE (~34us/SB):
f32r hi casts + fp8 hi casts + exp + er cast + xar cast on ScalarE,
lo subtract + fp8 lo cast split between DVE and GpSimd.
"""
import numpy as np
from contextlib import ExitStack

import concourse.bass as bass
import concourse.bacc as bacc
import concourse.tile as tile
from concourse import mybir
from concourse.bass_utils import run_bass_kernel_spmd
from concourse.masks import make_identity

P = 128
SEQ = 8192
DIM = 1024
NCORES = 8
AUG = 4      # extra columns on X_aug: [ones, 0, 0, 0]
SBN = 4      # key n-tiles (of 128) per flash super-block

F32 = mybir.dt.float32
F32R = mybir.dt.float32r
BF16 = mybir.dt.bfloat16
F8 = mybir.dt.float8e4
EXP = mybir.ActivationFunctionType.Exp
ALU = mybir.AluOpType
AXX = mybir.AxisListType.X
DR = mybir.MatmulPerfMode.DoubleRow

# fp8 cross-term scales (product of each pair = 1.0)
S_LX = 512.0      # lo(X) * 2^9   (stationary, term1)
S_HA = 1.0 / 512.0  # hi(A) * 2^-9  (moving, term1)
S_HX = 1.0 / 16.0   # hi(X) * 2^-4  (stationary, term2)
S_LA = 16.0         # lo(A) * 2^4   (moving, term2)


def _chunks(total, step=512):
    return [(lo, min(lo + step, total)) for lo in range(0, total, step)]


def build_core_kernel(S, D, B, sbn=SBN, aug=AUG):
    """One core's kernel: query rows block of size B, full S keys."""
    KT = D // P      # contraction tiles over D
    NT = S // P      # key tiles
    MT = B // P      # query tiles (per core)
    NSB = NT // sbn  # super-blocks
    NPAIR = KT // 2  # DoubleRow contraction pairs
    XAW = D + aug
    assert NT % sbn == 0 and B % P == 0 and D % P == 0 and MT <= P and KT % 2 == 0

    nc = bacc.Bacc("TRN2", target_bir_lowering=False, debug=False)
    xtj = nc.dram_tensor("xtj", [NT, P, D], F32, kind="ExternalInput")
    xa = nc.dram_tensor("xa", [S, XAW], BF16, kind="ExternalInput")
    wqs = nc.dram_tensor("wqs", [D, D], F32, kind="ExternalInput")
    wkt = nc.dram_tensor("wkt", [D, D], F32, kind="ExternalInput")
    xit = nc.dram_tensor("xit", [D, B], F32, kind="ExternalInput")
    out = nc.dram_tensor("out", [B, D], F32, kind="ExternalOutput")

    def pair_st(t, u):
        # stationary fp8 pair view [P, 2, P] of a [P, D] tile, pair u
        return t[:, u * 2 * P:(u + 1) * 2 * P].rearrange("p (i m) -> p i m", i=2)

    with tile.TileContext(nc) as tc, ExitStack() as ctx:
        pers = ctx.enter_context(tc.tile_pool(name="pers", bufs=1))
        aith = [pers.tile([P, B], F32R, name=f"aith{k}") for k in range(KT)]
        h8a = pers.tile([P, KT, B], F8, name="h8a")
        l8a = pers.tile([P, KT, B], F8, name="l8a")
        gm = pers.tile([P, B], F32, name="gm")
        mxbc = pers.tile([P, B], F32, name="mxbc")
        ident = pers.tile([P, P], F32, name="ident")
        make_identity(nc, ident[:])
        nc.gpsimd.memset(gm[:], -1e30)

        # ---- phase 0: T1 = Wqs^T X_i^T ; A^T = Wk T1 (per B-half) ----
        with ExitStack() as p0:
            t1p = p0.enter_context(tc.tile_pool(name="t1p", bufs=1))
            wp = p0.enter_context(tc.tile_pool(name="wp", bufs=2))
            xip = p0.enter_context(tc.tile_pool(name="xip", bufs=1))
            auxp = p0.enter_context(tc.tile_pool(name="auxp", bufs=2))
            ps0 = p0.enter_context(tc.tile_pool(name="ps0", bufs=4, space="PSUM"))
            HB = 512
            hT1 = t1p.tile([P, KT, HB], F32R, name="hT1")
            lT1 = t1p.tile([P, KT, HB], F32R, name="lT1")
            for (lo, hi) in _chunks(B):
                # X_i^T half-split: hi/lo f32r
                xi_f = xip.tile([P, KT, HB], F32, name=f"xif{lo}", tag="xif")
                for g in range(KT):
                    nc.sync.dma_start(xi_f[:, g, :], xit.ap()[g * P:(g + 1) * P, lo:hi])
                hxi = xip.tile([P, KT, HB], F32R, name=f"hxi{lo}", tag="hxi")
                nc.scalar.copy(hxi[:], xi_f[:])
                nc.vector.tensor_sub(xi_f[:], xi_f[:], hxi[:].bitcast(F32))
                lxi = xip.tile([P, KT, HB], F32R, name=f"lxi{lo}", tag="lxi")
                nc.vector.tensor_copy(lxi[:], xi_f[:])

                # step1: T1[r-tile, half] = sum_g Wqs[g,:][:, r]^T X^T[g, half]
                for r in range(KT):
                    wq_f = wp.tile([P, KT, P], F32, name=f"wqf{lo}_{r}", tag="wqf")
                    for g in range(KT):
                        nc.sync.dma_start(wq_f[:, g, :], wqs.ap()[g * P:(g + 1) * P, r * P:(r + 1) * P])
                    hwq = wp.tile([P, KT, P], F32R, name=f"hwq{lo}_{r}", tag="hwq")
                    nc.scalar.copy(hwq[:], wq_f[:])
                    nc.vector.tensor_sub(wq_f[:], wq_f[:], hwq[:].bitcast(F32))
                    lwq = wp.tile([P, KT, P], F32R, name=f"lwq{lo}_{r}", tag="lwq")
                    nc.vector.tensor_copy(lwq[:], wq_f[:])
                    t1ps = ps0.tile([P, HB], F32, name=f"t1ps{lo}_{r}", tag="pm")
                    n = 3 * KT
                    i = 0
                    for g in range(KT):
                        for (la, rb) in ((hwq, hxi), (hwq, lxi), (lwq, hxi)):
                            nc.tensor.matmul(t1ps[:], la[:, g, :], rb[:, g, :],
                                             start=(i == 0), stop=(i == n - 1))
                            i += 1
                    nc.vector.tensor_copy(hT1[:, r, :], t1ps[:])
                    nc.vector.tensor_sub(t1ps[:], t1ps[:], hT1[:, r, :].bitcast(F32))
                    nc.vector.tensor_copy(lT1[:, r, :], t1ps[:])

                # step2: A^T[d-tile, half] = sum_r Wk[:, r][d, :] T1[r, half]
                # (reuses the step1 weight-split slots: same tags/shapes)
                for d in range(KT):
                    wk_f = wp.tile([P, KT, P], F32, name=f"wkf{lo}_{d}", tag="wqf")
                    for r in range(KT):
                        nc.sync.dma_start(wk_f[:, r, :], wkt.ap()[r * P:(r + 1) * P, d * P:(d + 1) * P])
                    hwk = wp.tile([P, KT, P], F32R, name=f"hwk{lo}_{d}", tag="hwq")
                    nc.scalar.copy(hwk[:], wk_f[:])
                    nc.vector.tensor_sub(wk_f[:], wk_f[:], hwk[:].bitcast(F32))
                    lwk = wp.tile([P, KT, P], F32R, name=f"lwk{lo}_{d}", tag="lwq")
                    nc.vector.tensor_copy(lwk[:], wk_f[:])
                    pa = ps0.tile([P, HB], F32, name=f"pa{lo}_{d}", tag="pm")
                    n = 3 * KT
                    i = 0
                    for r in range(KT):
                        for (la, rb) in ((hwk, hT1), (hwk, lT1), (lwk, hT1)):
                            nc.tensor.matmul(pa[:], la[:, r, :], rb[:, r, :],
                                             start=(i == 0), stop=(i == n - 1))
                            i += 1
                    nc.vector.tensor_copy(aith[d][:, lo:hi], pa[:])
                    nc.scalar.mul(h8a[:, d, lo:hi], aith[d][:, lo:hi].bitcast(F32), S_HA)
                    al_f = auxp.tile([P, HB], F32, name=f"alf{lo}_{d}", tag="alf")
                    nc.vector.tensor_sub(al_f[:], pa[:], aith[d][:, lo:hi].bitcast(F32))
                    nc.vector.tensor_scalar_mul(l8a[:, d, lo:hi], al_f[:], S_LA)

        # ---- flash over key super-blocks ----
        accp = ctx.enter_context(tc.tile_pool(name="accp", bufs=1))
        acc = [accp.tile([P, XAW], F32, name=f"acc{t}") for t in range(MT)]
        for t in range(MT):
            nc.gpsimd.memset(acc[t][:], 0.0)
        sp = ctx.enter_context(tc.tile_pool(name="sp", bufs=2 * sbn + 2))
        erp = ctx.enter_context(tc.tile_pool(name="erp", bufs=2 * sbn))
        xap = ctx.enter_context(tc.tile_pool(name="xap", bufs=3))
        xarp = ctx.enter_context(tc.tile_pool(name="xarp", bufs=2 * sbn))
        xtp = xap
        xthp = ctx.enter_context(tc.tile_pool(name="xthp", bufs=3))
        stat = ctx.enter_context(tc.tile_pool(name="stat", bufs=2))
        ps_s = ctx.enter_context(tc.tile_pool(name="ps_s", bufs=2, space="PSUM"))
        ps_o = ctx.enter_context(tc.tile_pool(name="ps_o", bufs=2, space="PSUM"))
        ps_t = ps_s

        # Software pipeline, one-super-block lag, tuned so the PE queue is
        # [S(s) | out(s-1) | transposes(s) | S(s+1) | ...] with no waits:
        # out(s-1)'s operands (er/xar/corr) were finished during S(s)'s burst,
        # and the gm column-maxes feeding transposes(s) finish during
        # out(s-1). The exp chain for s runs on DVE/ScalarE under S(s+1).
        # E and X_aug are bf16 for the out matmul (same 1 cyc/row as f32r,
        # half the SBUF; E's 2^-9 rounding cancels through the ones-column
        # denominator, X_aug's is ~2e-3 of |x| — both far under budget).
        # X_aug ships from the host already in bf16, so no on-device cast.
        def prep_block(s):
            js = list(range(s * sbn, (s + 1) * sbn))
            xsplit = []
            for j in js:
                xt_t = xtp.tile([P, XAW], F32, name=f"xt{j}", tag="stg")
                nc.sync.dma_start(xt_t[:, :D], xtj.ap()[j])
                xth = xthp.tile([P, D], F32R, name=f"xth{j}", tag="xth")
                nc.scalar.copy(xth[:], xt_t[:, :D])
                nc.vector.tensor_sub(xt_t[:, :D], xt_t[:, :D], xth[:].bitcast(F32))
                l8x = xthp.tile([P, D], F8, name=f"l8x{j}", tag="l8x")
                nc.vector.tensor_scalar_mul(l8x[:], xt_t[:, :D], S_LX)
                h8x = xthp.tile([P, D], F8, name=f"h8x{j}", tag="h8x")
                nc.scalar.mul(h8x[:], xth[:].bitcast(F32), S_HX)
                xsplit.append((xth, l8x, h8x))
            return xsplit

        def s_burst(s, xsplit):
            ssb = []
            for idx, j in enumerate(range(s * sbn, (s + 1) * sbn)):
                xth, l8x, h8x = xsplit[idx]
                s_t = sp.tile([P, B], F32, name=f"s{j}", tag="s")
                pss = [ps_s.tile([P, 512], F32, name=f"pss{j}_{c}", tag="pss")
                       for c in range(2)]
                # f32r hh pass, both chunks back-to-back (same PE mode)
                for c, (lo, hi) in enumerate(_chunks(B)):
                    for k in range(KT):
                        nc.tensor.matmul(pss[c][:], xth[:, k * P:(k + 1) * P],
                                         aith[k][:, lo:hi], start=(k == 0), stop=(k == KT - 1))
                # fp8 DoubleRow cross passes: each accumulates onto the
                # closed f32r group via has_written (start=False); stop is
                # sim-only bookkeeping so every DR matmul closes itself
                for c, (lo, hi) in enumerate(_chunks(B)):
                    for u in range(NPAIR):
                        nc.tensor.matmul(pss[c][:], pair_st(l8x, u),
                                         h8a[:, 2 * u:2 * u + 2, lo:hi],
                                         start=False, stop=True, perf_mode=DR,
                                         skip_group_check=True)
                    for u in range(NPAIR):
                        nc.tensor.matmul(pss[c][:], pair_st(h8x, u),
                                         l8a[:, 2 * u:2 * u + 2, lo:hi],
                                         start=False, stop=True, perf_mode=DR,
                                         skip_group_check=True)
                for c, (lo, hi) in enumerate(_chunks(B)):
                    nc.scalar.copy(s_t[:, lo:hi], pss[c][:])
                    nc.vector.tensor_max(gm[:, lo:hi], gm[:, lo:hi], pss[c][:])
                ssb.append(s_t)
            return ssb

        def stats_block(s, omx):
            # per-query-column running max (transpose-reduce gm chunks)
            nmx = stat.tile([P, MT], F32, name=f"nmx{s}", tag="nmx")
            corr = stat.tile([P, MT], F32, name=f"corr{s}", tag="corr")
            for c in range(MT):
                pt = ps_t.tile([P, P], F32, name=f"pt{s}_{c}", tag="pss")
                nc.tensor.transpose(pt[:], gm[:, c * P:(c + 1) * P], ident[:])
                nc.vector.reduce_max(nmx[:, c:c + 1], pt[:], axis=AXX)
            if omx is None:
                nc.vector.memset(corr[:], 0.0)
            else:
                dmx = stat.tile([P, MT], F32, name=f"dmx{s}", tag="dmx")
                nc.vector.tensor_sub(dmx[:], omx[:], nmx[:])
                nc.scalar.activation(corr[:], dmx[:], EXP)

            # broadcast nmx (query-major) -> mxbc [P, B] (key-major free)
            ptb = ps_t.tile([P, P], F32, name=f"ptb{s}", tag="pss")
            nc.tensor.transpose(ptb[:MT, :], nmx[:], ident[:])
            mtmp = stat.tile([MT, P], F32, name=f"mtmp{s}", tag="mtmp")
            nc.scalar.copy(mtmp[:], ptb[:MT, :])
            mrow = stat.tile([1, B], F32, name=f"mrow{s}", tag="mrow", bufs=1)
            nc.sync.dma_start(mrow[:].rearrange("a (b c) -> a b c", b=MT), mtmp[:])
            nc.gpsimd.partition_broadcast(mxbc[:], mrow[:])
            return nmx, corr

        def exp_block(s, ssb):
            # E = exp(S - max), exp writes bf16 er (out dtype converts)
            ers = []
            for idx, s_t in enumerate(ssb):
                nc.vector.tensor_sub(s_t[:], s_t[:], mxbc[:])
                er_t = erp.tile([P, B], BF16, name=f"er{s}_{idx}", tag="er")
                nc.scalar.activation(er_t[:], s_t[:], EXP)
                ers.append(er_t)
            return ers

        def xar_block(s):
            xar = []
            for j in range(s * sbn, (s + 1) * sbn):
                xa_t = xarp.tile([P, XAW], BF16, name=f"xa{j}", tag="xar")
                nc.sync.dma_start(xa_t[:], xa.ap()[j * P:(j + 1) * P, :])
                xar.append(xa_t)
            return xar

        def out_block(s, ers, xar, corr):
            # out accumulation: acc = acc*corr + E^T @ X_aug (f32r burst);
            # 3 matmul streams into bank-aligned slices of one PSUM tile,
            # then a single fused rescale-accumulate per query tile
            for t in range(MT):
                po = ps_o.tile([P, XAW], F32, name=f"po{s}_{t}", tag="po")
                # idx outer so the 3 column chunks reuse one stationary
                # operand back-to-back (LDWEIGHTS locality); each chunk's
                # PSUM accumulation group still spans idx 0..sbn-1
                for idx in range(sbn):
                    er = ers[idx][:]
                    for (lo, hi) in _chunks(XAW):
                        nc.tensor.matmul(po[:, lo:hi], er[:, t * P:(t + 1) * P],
                                         xar[idx][:, lo:hi], start=(idx == 0), stop=(idx == sbn - 1))
                nc.vector.scalar_tensor_tensor(acc[t][:], acc[t][:],
                                               corr[:, t:t + 1], po[:],
                                               op0=ALU.mult, op1=ALU.add)

        omx = None
        prev = None  # (ers, xar, corr) of block s-1
        xsplit = prep_block(0)
        for s in range(NSB):
            ssb = s_burst(s, xsplit)
            if s + 1 < NSB:
                xsplit = prep_block(s + 1)
            if prev is not None:
                out_block(s - 1, *prev)
            nmx, corr = stats_block(s, omx)
            omx = nmx
            ers = exp_block(s, ssb)
            xar = xar_block(s)
            prev = (ers, xar, corr)
        out_block(NSB - 1, *prev)

        # ---- finalize: divide by the ones-column sums, write out ----
        if True:
            for t in range(MT):
                rc = stat.tile([P, 1], F32, name=f"rc{t}", tag="rc")
                nc.vector.reciprocal(rc[:], acc[t][:, D:D + 1])
                nc.vector.tensor_scalar_mul(acc[t][:, 0:D], acc[t][:, 0:D], rc[:])
                nc.sync.dma_start(out.ap()[t * P:(t + 1) * P, :], acc[t][:, 0:D])

    nc.compile()
    return nc


def prep_inputs(X, Wq, Wk, S, D, n_cores, aug=AUG):
    B = S // n_cores
    NT = S // P
    KT = D // P
    X = np.ascontiguousarray(X, np.float32)
    scale = np.float32(1.0 / np.sqrt(D))
    xtj = np.ascontiguousarray(
        X.reshape(NT, P, KT, P).transpose(0, 3, 2, 1).reshape(NT, P, D))
    import ml_dtypes
    xa = np.zeros((S, D + aug), ml_dtypes.bfloat16)
    xa[:, :D] = X.astype(ml_dtypes.bfloat16)
    xa[:, D] = 1.0
    wqs = np.ascontiguousarray(np.asarray(Wq, np.float32) * scale)
    wkt = np.ascontiguousarray(np.asarray(Wk, np.float32).T)
    xt = X.T
    in_maps = []
    for i in range(n_cores):
        in_maps.append({
            "xtj": xtj, "xa": xa, "wqs": wqs, "wkt": wkt,
            "xit": np.ascontiguousarray(xt[:, i * B:(i + 1) * B]),
        })
    return in_maps


_CACHE = {}


def _get_kernel(S, D, B, sbn):
    key = (S, D, B, sbn)
    if key not in _CACHE:
        _CACHE[key] = build_core_kernel(S, D, B, sbn=sbn)
    return _CACHE[key]


def kernel(inputs, weight_query, weight_key):
    S, D = inputs.shape
    assert (S, D) == (SEQ, DIM)
    B = S // NCORES
    nc = _get_kernel(S, D, B, SBN)
    in_maps = prep_inputs(inputs, weight_query, weight_key, S, D, NCORES)
    res = run_bass_kernel_spmd(nc, in_maps, core_ids=list(range(NCORES)))
    return np.concatenate([res.results[i]["out"] for i in range(NCORES)], axis=0)


if __name__ == "__main__":
    rng = np.random.default_rng(0)
    X = rng.standard_normal((SEQ, DIM), dtype=np.float32)
    Wq = rng.standard_normal((DIM, DIM), dtype=np.float32)
    Wk = rng.standard_normal((DIM, DIM), dtype=np.float32)
    out = kernel(X, Wq, Wk)
    print(out.shape, out.dtype)


# revision 21
# speedup vs baseline: 1.8004x; 1.0796x over previous
"""Self-attention kernel for TRN2: out = softmax(X Wq (X Wk)^T / sqrt(D)) @ X.

Strategy (8-way sequence parallelism over query rows):
  scores = (X Wqs)(X Wk)^T = X A  with  A^T = Wk (Wqs^T X_i^T), Wqs = Wq/sqrt(D)
so K and M = Wqs Wk^T are never materialized. Each core i handles query rows
[i*B, (i+1)*B):
  phase 0 (two streamed GEMM steps, per B-half):
    step1: T1 = Wqs^T X_i^T  (3-pass f32r hi/lo: hh+hl+lh — T1 errors amplify
           by ~D into the logits, so crosses must stay near-exact)
    step2: A^T = Wk T1       (3-pass f32r; same sensitivity via sqrt(D)*|X|)
    outputs: A^T as f32r hi tiles (aith) + e4m3 cross operands
           h8a = hi(A)*2^-9, l8a = lo(A)*2^4 in DoubleRow pair layout.
  flash:   stream key blocks j; logits S^T_j = X_j A in key-major layout as
           f32r hh pass + TWO fp8 DoubleRow cross passes (0.5 cyc/row):
             term1 = lo(X)*2^9 (stationary) x hi(A)*2^-9 (moving)
             term2 = hi(X)*2^-4 x lo(A)*2^4
           running column-max via PE transpose + reduce; E = exp(S - max)
           f32r; second matmul E^T-slices @ X_aug in f32r; ones-column in
           X_aug accumulates the softmax denominator; fused
           rescale-accumulate (acc = acc*corr + psum) on DVE; final divide.

Numerics: logits need ~17-bit abs precision (std ~1024, near-tie rows
amplify errors through softmax). f32r hi/lo split leaves cross terms at
~2^-12 relative, so quantizing THOSE to e4m3 (3-bit mantissa) only adds
~7e-3 logit noise — well under the ~0.04 budget. The P@X matmul only needs
~1e-3 relative, so a single f32r pass is safe there.

Engine placement tuned so DVE (~27us/SB) stays under PE (~34us/SB):
f32r hi casts + fp8 hi casts + exp + er cast + xar cast on ScalarE,
lo subtract + fp8 lo cast split between DVE and GpSimd.
"""
import numpy as np
from contextlib import ExitStack

import concourse.bass as bass
import concourse.bacc as bacc
import concourse.tile as tile
from concourse import mybir
from concourse.bass_utils import run_bass_kernel_spmd
from concourse.masks import make_identity

P = 128
SEQ = 8192
DIM = 1024
NCORES = 8
AUG = 4      # extra columns on X_aug: [ones, 0, 0, 0]
SBN = 4      # key n-tiles (of 128) per flash super-block

F32 = mybir.dt.float32
F32R = mybir.dt.float32r
BF16 = mybir.dt.bfloat16
F8 = mybir.dt.float8e4
EXP = mybir.ActivationFunctionType.Exp
ALU = mybir.AluOpType
AXX = mybir.AxisListType.X
DR = mybir.MatmulPerfMode.DoubleRow

# fp8 cross-term scales (product of each pair = 1.0)
S_LX = 512.0      # lo(X) * 2^9   (stationary, term1)
S_HA = 1.0 / 512.0  # hi(A) * 2^-9  (moving, term1)
S_HX = 1.0 / 16.0   # hi(X) * 2^-4  (stationary, term2)
S_LA = 16.0         # lo(A) * 2^4   (moving, term2)


def _chunks(total, step=512):
    return [(lo, min(lo + step, total)) for lo in range(0, total, step)]


def build_core_kernel(S, D, B, sbn=SBN, aug=AUG):
    """One core's kernel: query rows block of size B, full S keys."""
    KT = D // P      # contraction tiles over D
    NT = S // P      # key tiles
    MT = B // P      # query tiles (per core)
    NSB = NT // sbn  # super-blocks
    NPAIR = KT // 2  # DoubleRow contraction pairs
    XAW = D + aug
    assert NT % sbn == 0 and B % P == 0 and D % P == 0 and MT <= P and KT % 2 == 0

    nc = bacc.Bacc("TRN2", target_bir_lowering=False, debug=False)
    xtj = nc.dram_tensor("xtj", [NT, P, D], F32, kind="ExternalInput")
    xa = nc.dram_tensor("xa", [S, XAW], BF16, kind="ExternalInput")
    wqs = nc.dram_tensor("wqs", [D, D], F32, kind="ExternalInput")
    wkt = nc.dram_tensor("wkt", [D, D], F32, kind="ExternalInput")
    xit = nc.dram_tensor("xit", [D, B], F32, kind="ExternalInput")
    out = nc.dram_tensor("out", [B, D], F32, kind="ExternalOutput")

    def pair_st(t, u):
        # stationary fp8 pair view [P, 2, P] of a [P, D] tile, pair u
        return t[:, u * 2 * P:(u + 1) * 2 * P].rearrange("p (i m) -> p i m", i=2)

    with tile.TileContext(nc) as tc, ExitStack() as ctx:
        pers = ctx.enter_context(tc.tile_pool(name="pers", bufs=1))
        aith = [pers.tile([P, B], F32R, name=f"aith{k}") for k in range(KT)]
        h8a = pers.tile([P, KT, B], F8, name="h8a")
        l8a = pers.tile([P, KT, B], F8, name="l8a")
        gm = pers.tile([P, B], F32, name="gm")
        mxbc = pers.tile([P, B], F32, name="mxbc")
        ident = pers.tile([P, P], F32, name="ident")
        make_identity(nc, ident[:])
        nc.gpsimd.memset(gm[:], -1e30)

        # ---- phase 0: T1 = Wqs^T X_i^T ; A^T = Wk T1 (per B-half) ----
        with ExitStack() as p0:
            t1p = p0.enter_context(tc.tile_pool(name="t1p", bufs=1))
            wp = p0.enter_context(tc.tile_pool(name="wp", bufs=2))
            xip = p0.enter_context(tc.tile_pool(name="xip", bufs=1))
            auxp = p0.enter_context(tc.tile_pool(name="auxp", bufs=2))
            ps0 = p0.enter_context(tc.tile_pool(name="ps0", bufs=4, space="PSUM"))
            HB = 512
            hT1 = t1p.tile([P, KT, HB], F32R, name="hT1")
            lT1 = t1p.tile([P, KT, HB], F32R, name="lT1")
            for (lo, hi) in _chunks(B):
                # X_i^T half-split: hi/lo f32r, pipelined per g-tile so the
                # first step1 matmuls start as soon as g=0 lands
                xi_f = xip.tile([P, KT, HB], F32, name=f"xif{lo}", tag="xif")
                hxi = xip.tile([P, KT, HB], F32R, name=f"hxi{lo}", tag="hxi")
                lxi = xip.tile([P, KT, HB], F32R, name=f"lxi{lo}", tag="lxi")
                for g in range(KT):
                    nc.sync.dma_start(xi_f[:, g, :], xit.ap()[g * P:(g + 1) * P, lo:hi])
                    nc.scalar.copy(hxi[:, g, :], xi_f[:, g, :])
                    nc.vector.tensor_sub(xi_f[:, g, :], xi_f[:, g, :], hxi[:, g, :].bitcast(F32))
                    nc.vector.tensor_copy(lxi[:, g, :], xi_f[:, g, :])

                # step1: T1[r-tile, half] = sum_g Wqs[g,:][:, r]^T X^T[g, half]
                for r in range(KT):
                    wq_f = wp.tile([P, KT, P], F32, name=f"wqf{lo}_{r}", tag="wqf")
                    for g in range(KT):
                        nc.sync.dma_start(wq_f[:, g, :], wqs.ap()[g * P:(g + 1) * P, r * P:(r + 1) * P])
                    hwq = wp.tile([P, KT, P], F32R, name=f"hwq{lo}_{r}", tag="hwq")
                    nc.scalar.copy(hwq[:], wq_f[:])
                    nc.vector.tensor_sub(wq_f[:], wq_f[:], hwq[:].bitcast(F32))
                    lwq = wp.tile([P, KT, P], F32R, name=f"lwq{lo}_{r}", tag="lwq")
                    nc.vector.tensor_copy(lwq[:], wq_f[:])
                    t1ps = ps0.tile([P, HB], F32, name=f"t1ps{lo}_{r}", tag="pm")
                    n = 3 * KT
                    i = 0
                    for g in range(KT):
                        for (la, rb) in ((hwq, hxi), (hwq, lxi), (lwq, hxi)):
                            nc.tensor.matmul(t1ps[:], la[:, g, :], rb[:, g, :],
                                             start=(i == 0), stop=(i == n - 1))
                            i += 1
                    nc.vector.tensor_copy(hT1[:, r, :], t1ps[:])
                    nc.vector.tensor_sub(t1ps[:], t1ps[:], hT1[:, r, :].bitcast(F32))
                    nc.vector.tensor_copy(lT1[:, r, :], t1ps[:])

                # step2: A^T[d-tile, half] = sum_r Wk[:, r][d, :] T1[r, half]
                # (reuses the step1 weight-split slots: same tags/shapes)
                for d in range(KT):
                    wk_f = wp.tile([P, KT, P], F32, name=f"wkf{lo}_{d}", tag="wqf")
                    for r in range(KT):
                        nc.sync.dma_start(wk_f[:, r, :], wkt.ap()[r * P:(r + 1) * P, d * P:(d + 1) * P])
                    hwk = wp.tile([P, KT, P], F32R, name=f"hwk{lo}_{d}", tag="hwq")
                    nc.scalar.copy(hwk[:], wk_f[:])
                    nc.vector.tensor_sub(wk_f[:], wk_f[:], hwk[:].bitcast(F32))
                    lwk = wp.tile([P, KT, P], F32R, name=f"lwk{lo}_{d}", tag="lwq")
                    nc.vector.tensor_copy(lwk[:], wk_f[:])
                    pa = ps0.tile([P, HB], F32, name=f"pa{lo}_{d}", tag="pm")
                    n = 3 * KT
                    i = 0
                    for r in range(KT):
                        for (la, rb) in ((hwk, hT1), (hwk, lT1), (lwk, hT1)):
                            nc.tensor.matmul(pa[:], la[:, r, :], rb[:, r, :],
                                             start=(i == 0), stop=(i == n - 1))
                            i += 1
                    nc.vector.tensor_copy(aith[d][:, lo:hi], pa[:])
                    nc.scalar.mul(h8a[:, d, lo:hi], aith[d][:, lo:hi].bitcast(F32), S_HA)
                    al_f = auxp.tile([P, HB], F32, name=f"alf{lo}_{d}", tag="alf")
                    nc.vector.tensor_sub(al_f[:], pa[:], aith[d][:, lo:hi].bitcast(F32))
                    nc.vector.tensor_scalar_mul(l8a[:, d, lo:hi], al_f[:], S_LA)

        # ---- flash over key super-blocks ----
        accp = ctx.enter_context(tc.tile_pool(name="accp", bufs=1))
        acc = [accp.tile([P, XAW], F32, name=f"acc{t}") for t in range(MT)]
        for t in range(MT):
            nc.gpsimd.memset(acc[t][:], 0.0)
        sp = ctx.enter_context(tc.tile_pool(name="sp", bufs=2 * sbn + 2))
        erp = ctx.enter_context(tc.tile_pool(name="erp", bufs=2 * sbn))
        xap = ctx.enter_context(tc.tile_pool(name="xap", bufs=3))
        xarp = ctx.enter_context(tc.tile_pool(name="xarp", bufs=2 * sbn))
        xtp = xap
        xthp = ctx.enter_context(tc.tile_pool(name="xthp", bufs=3))
        stat = ctx.enter_context(tc.tile_pool(name="stat", bufs=2))
        ps_s = ctx.enter_context(tc.tile_pool(name="ps_s", bufs=2, space="PSUM"))
        ps_o = ctx.enter_context(tc.tile_pool(name="ps_o", bufs=2, space="PSUM"))
        ps_t = ps_s

        # Software pipeline, one-super-block lag, tuned so the PE queue is
        # [S(s) | out(s-1) | transposes(s) | S(s+1) | ...] with no waits:
        # out(s-1)'s operands (er/xar/corr) were finished during S(s)'s burst,
        # and the gm column-maxes feeding transposes(s) finish during
        # out(s-1). The exp chain for s runs on DVE/ScalarE under S(s+1).
        # E and X_aug are bf16 for the out matmul (same 1 cyc/row as f32r,
        # half the SBUF; E's 2^-9 rounding cancels through the ones-column
        # denominator, X_aug's is ~2e-3 of |x| — both far under budget).
        # X_aug ships from the host already in bf16, so no on-device cast.
        def prep_block(s):
            js = list(range(s * sbn, (s + 1) * sbn))
            xsplit = []
            for j in js:
                xt_t = xtp.tile([P, XAW], F32, name=f"xt{j}", tag="stg")
                nc.sync.dma_start(xt_t[:, :D], xtj.ap()[j])
                xth = xthp.tile([P, D], F32R, name=f"xth{j}", tag="xth")
                nc.scalar.copy(xth[:], xt_t[:, :D])
                nc.vector.tensor_sub(xt_t[:, :D], xt_t[:, :D], xth[:].bitcast(F32))
                l8x = xthp.tile([P, D], F8, name=f"l8x{j}", tag="l8x")
                nc.vector.tensor_scalar_mul(l8x[:], xt_t[:, :D], S_LX)
                h8x = xthp.tile([P, D], F8, name=f"h8x{j}", tag="h8x")
                nc.scalar.mul(h8x[:], xth[:].bitcast(F32), S_HX)
                xsplit.append((xth, l8x, h8x))
            return xsplit

        def s_burst(s, xsplit):
            ssb = []
            for idx, j in enumerate(range(s * sbn, (s + 1) * sbn)):
                xth, l8x, h8x = xsplit[idx]
                s_t = sp.tile([P, B], F32, name=f"s{j}", tag="s")
                pss = [ps_s.tile([P, 512], F32, name=f"pss{j}_{c}", tag="pss")
                       for c in range(2)]
                # f32r hh pass, both chunks back-to-back (same PE mode)
                for c, (lo, hi) in enumerate(_chunks(B)):
                    for k in range(KT):
                        nc.tensor.matmul(pss[c][:], xth[:, k * P:(k + 1) * P],
                                         aith[k][:, lo:hi], start=(k == 0), stop=(k == KT - 1))
                # fp8 DoubleRow cross passes: each accumulates onto the
                # closed f32r group via has_written (start=False); stop is
                # sim-only bookkeeping so every DR matmul closes itself.
                # Chunk 0 drains (copy + running max) while chunk 1's fp8
                # matmuls stream, so the stats transposes can start the
                # moment the burst ends.
                for c, (lo, hi) in enumerate(_chunks(B)):
                    for u in range(NPAIR):
                        nc.tensor.matmul(pss[c][:], pair_st(l8x, u),
                                         h8a[:, 2 * u:2 * u + 2, lo:hi],
                                         start=False, stop=True, perf_mode=DR,
                                         skip_group_check=True)
                    for u in range(NPAIR):
                        nc.tensor.matmul(pss[c][:], pair_st(h8x, u),
                                         l8a[:, 2 * u:2 * u + 2, lo:hi],
                                         start=False, stop=True, perf_mode=DR,
                                         skip_group_check=True)
                    nc.scalar.copy(s_t[:, lo:hi], pss[c][:])
                    nc.vector.tensor_max(gm[:, lo:hi], gm[:, lo:hi], pss[c][:])
                ssb.append(s_t)
            return ssb

        def stats_block(s, omx):
            # per-query-column running max (transpose-reduce gm chunks)
            nmx = stat.tile([P, MT], F32, name=f"nmx{s}", tag="nmx")
            corr = stat.tile([P, MT], F32, name=f"corr{s}", tag="corr")
            for c in range(MT):
                pt = ps_t.tile([P, P], F32, name=f"pt{s}_{c}", tag="pss")
                nc.tensor.transpose(pt[:], gm[:, c * P:(c + 1) * P], ident[:])
                nc.vector.reduce_max(nmx[:, c:c + 1], pt[:], axis=AXX)
            if omx is None:
                nc.vector.memset(corr[:], 0.0)
            else:
                dmx = stat.tile([P, MT], F32, name=f"dmx{s}", tag="dmx")
                nc.vector.tensor_sub(dmx[:], omx[:], nmx[:])
                nc.scalar.activation(corr[:], dmx[:], EXP)

            # broadcast nmx (query-major) -> mxbc [P, B] (key-major free)
            ptb = ps_t.tile([P, P], F32, name=f"ptb{s}", tag="pss")
            nc.tensor.transpose(ptb[:MT, :], nmx[:], ident[:])
            mtmp = stat.tile([MT, P], F32, name=f"mtmp{s}", tag="mtmp")
            nc.scalar.copy(mtmp[:], ptb[:MT, :])
            mrow = stat.tile([1, B], F32, name=f"mrow{s}", tag="mrow", bufs=1)
            nc.sync.dma_start(mrow[:].rearrange("a (b c) -> a b c", b=MT), mtmp[:])
            nc.gpsimd.partition_broadcast(mxbc[:], mrow[:])
            return nmx, corr

        def exp_block(s, ssb):
            # E = exp(S - max), exp writes bf16 er (out dtype converts)
            ers = []
            for idx, s_t in enumerate(ssb):
                nc.vector.tensor_sub(s_t[:], s_t[:], mxbc[:])
                er_t = erp.tile([P, B], BF16, name=f"er{s}_{idx}", tag="er")
                nc.scalar.activation(er_t[:], s_t[:], EXP)
                ers.append(er_t)
            return ers

        def xar_block(s):
            xar = []
            for j in range(s * sbn, (s + 1) * sbn):
                xa_t = xarp.tile([P, XAW], BF16, name=f"xa{j}", tag="xar")
                nc.sync.dma_start(xa_t[:], xa.ap()[j * P:(j + 1) * P, :])
                xar.append(xa_t)
            return xar

        def out_block(s, ers, xar, corr):
            # out accumulation: acc = acc*corr + E^T @ X_aug (f32r burst);
            # 3 matmul streams into bank-aligned slices of one PSUM tile,
            # then a single fused rescale-accumulate per query tile
            for t in range(MT):
                po = ps_o.tile([P, XAW], F32, name=f"po{s}_{t}", tag="po")
                # idx outer so the 3 column chunks reuse one stationary
                # operand back-to-back (LDWEIGHTS locality); each chunk's
                # PSUM accumulation group still spans idx 0..sbn-1
                for idx in range(sbn):
                    er = ers[idx][:]
                    for (lo, hi) in _chunks(XAW):
                        nc.tensor.matmul(po[:, lo:hi], er[:, t * P:(t + 1) * P],
                                         xar[idx][:, lo:hi], start=(idx == 0), stop=(idx == sbn - 1))
                nc.vector.scalar_tensor_tensor(acc[t][:], acc[t][:],
                                               corr[:, t:t + 1], po[:],
                                               op0=ALU.mult, op1=ALU.add)

        omx = None
        prev = None  # (ers, xar, corr) of block s-1
        xsplit = prep_block(0)
        for s in range(NSB):
            ssb = s_burst(s, xsplit)
            if s + 1 < NSB:
                xsplit = prep_block(s + 1)
            # stats(s) on PE right after the burst (its gm maxes already
            # drained), so the max-broadcast/exp chain overlaps out(s-1)
            nmx, corr = stats_block(s, omx)
            omx = nmx
            if prev is not None:
                out_block(s - 1, *prev)
            ers = exp_block(s, ssb)
            xar = xar_block(s)
            prev = (ers, xar, corr)
        out_block(NSB - 1, *prev)

        # ---- finalize: divide by the ones-column sums, write out ----
        if True:
            for t in range(MT):
                rc = stat.tile([P, 1], F32, name=f"rc{t}", tag="rc")
                nc.vector.reciprocal(rc[:], acc[t][:, D:D + 1])
                nc.vector.tensor_scalar_mul(acc[t][:, 0:D], acc[t][:, 0:D], rc[:])
                nc.sync.dma_start(out.ap()[t * P:(t + 1) * P, :], acc[t][:, 0:D])

    nc.compile()
    return nc


def prep_inputs(X, Wq, Wk, S, D, n_cores, aug=AUG):
    B = S // n_cores
    NT = S // P
    KT = D // P
    X = np.ascontiguousarray(X, np.float32)
    scale = np.float32(1.0 / np.sqrt(D))
    xtj = np.ascontiguousarray(
        X.reshape(NT, P, KT, P).transpose(0, 3, 2, 1).reshape(NT, P, D))
    import ml_dtypes
    xa = np.zeros((S, D + aug), ml_dtypes.bfloat16)
    xa[:, :D] = X.astype(ml_dtypes.bfloat16)
    xa[:, D] = 1.0
    wqs = np.ascontiguousarray(np.asarray(Wq, np.float32) * scale)
    wkt = np.ascontiguousarray(np.asarray(Wk, np.float32).T)
    xt = X.T
    in_maps = []
    for i in range(n_cores):
        in_maps.append({
            "xtj": xtj, "xa": xa, "wqs": wqs, "wkt": wkt,
            "xit": np.ascontiguousarray(xt[:, i * B:(i + 1) * B]),
        })
    return in_maps


_CACHE = {}


def _get_kernel(S, D, B, sbn):
    key = (S, D, B, sbn)
    if key not in _CACHE:
        _CACHE[key] = build_core_kernel(S, D, B, sbn=sbn)
    return _CACHE[key]


def kernel(inputs, weight_query, weight_key):
    S, D = inputs.shape
    assert (S, D) == (SEQ, DIM)
    B = S // NCORES
    nc = _get_kernel(S, D, B, SBN)
    in_maps = prep_inputs(inputs, weight_query, weight_key, S, D, NCORES)
    res = run_bass_kernel_spmd(nc, in_maps, core_ids=list(range(NCORES)))
    return np.concatenate([res.results[i]["out"] for i in range(NCORES)], axis=0)


if __name__ == "__main__":
    rng = np.random.default_rng(0)
    X = rng.standard_normal((SEQ, DIM), dtype=np.float32)
    Wq = rng.standard_normal((DIM, DIM), dtype=np.float32)
    Wk = rng.standard_normal((DIM, DIM), dtype=np.float32)
    out = kernel(X, Wq, Wk)
    print(out.shape, out.dtype)


# revision 23
# speedup vs baseline: 1.8570x; 1.0314x over previous
"""Self-attention kernel for TRN2: out = softmax(X Wq (X Wk)^T / sqrt(D)) @ X.

Strategy (8-way sequence parallelism over query rows):
  scores = (X Wqs)(X Wk)^T = X A  with  A^T = Wk (Wqs^T X_i^T), Wqs = Wq/sqrt(D)
so K and M = Wqs Wk^T are never materialized. Each core i handles query rows
[i*B, (i+1)*B):
  phase 0 (two streamed GEMM steps, per B-half):
    step1: T1 = Wqs^T X_i^T  (3-pass f32r hi/lo: hh+hl+lh — T1 errors amplify
           by ~D into the logits, so crosses must stay near-exact)
    step2: A^T = Wk T1       (3-pass f32r; same sensitivity via sqrt(D)*|X|)
    outputs: A^T as f32r hi tiles (aith) + e4m3 cross operands
           h8a = hi(A)*2^-9, l8a = lo(A)*2^4 in DoubleRow pair layout.
  flash:   stream key blocks j; logits S^T_j = X_j A in key-major layout as
           f32r hh pass + TWO fp8 DoubleRow cross passes (0.5 cyc/row):
             term1 = lo(X)*2^9 (stationary) x hi(A)*2^-9 (moving)
             term2 = hi(X)*2^-4 x lo(A)*2^4
           running column-max via PE transpose + reduce; E = exp(S - max)
           f32r; second matmul E^T-slices @ X_aug in f32r; ones-column in
           X_aug accumulates the softmax denominator; fused
           rescale-accumulate (acc = acc*corr + psum) on DVE; final divide.

Numerics: logits need ~17-bit abs precision (std ~1024, near-tie rows
amplify errors through softmax). f32r hi/lo split leaves cross terms at
~2^-12 relative, so quantizing THOSE to e4m3 (3-bit mantissa) only adds
~7e-3 logit noise — well under the ~0.04 budget. The P@X matmul only needs
~1e-3 relative, so a single f32r pass is safe there.

Engine placement tuned so DVE (~27us/SB) stays under PE (~34us/SB):
f32r hi casts + fp8 hi casts + exp + er cast + xar cast on ScalarE,
lo subtract + fp8 lo cast split between DVE and GpSimd.
"""
import numpy as np
from contextlib import ExitStack

import concourse.bass as bass
import concourse.bacc as bacc
import concourse.tile as tile
from concourse import mybir
from concourse.bass_utils import run_bass_kernel_spmd
from concourse.masks import make_identity

P = 128
SEQ = 8192
DIM = 1024
NCORES = 8
AUG = 4      # extra columns on X_aug: [ones, 0, 0, 0]
SBN = 4      # key n-tiles (of 128) per flash super-block

F32 = mybir.dt.float32
F32R = mybir.dt.float32r
BF16 = mybir.dt.bfloat16
F8 = mybir.dt.float8e4
EXP = mybir.ActivationFunctionType.Exp
ALU = mybir.AluOpType
AXX = mybir.AxisListType.X
DR = mybir.MatmulPerfMode.DoubleRow

# fp8 cross-term scales (product of each pair = 1.0)
S_LX = 512.0      # lo(X) * 2^9   (stationary, term1)
S_HA = 1.0 / 512.0  # hi(A) * 2^-9  (moving, term1)
S_HX = 1.0 / 16.0   # hi(X) * 2^-4  (stationary, term2)
S_LA = 16.0         # lo(A) * 2^4   (moving, term2)


def _chunks(total, step=512):
    return [(lo, min(lo + step, total)) for lo in range(0, total, step)]


def build_core_kernel(S, D, B, sbn=SBN, aug=AUG):
    """One core's kernel: query rows block of size B, full S keys."""
    KT = D // P      # contraction tiles over D
    NT = S // P      # key tiles
    MT = B // P      # query tiles (per core)
    NSB = NT // sbn  # super-blocks
    NPAIR = KT // 2  # DoubleRow contraction pairs
    XAW = D + aug
    assert NT % sbn == 0 and B % P == 0 and D % P == 0 and MT <= P and KT % 2 == 0

    nc = bacc.Bacc("TRN2", target_bir_lowering=False, debug=False)
    xtj = nc.dram_tensor("xtj", [NT, P, D], F32, kind="ExternalInput")
    xa = nc.dram_tensor("xa", [S, XAW], BF16, kind="ExternalInput")
    wqs = nc.dram_tensor("wqs", [D, D], F32, kind="ExternalInput")
    wkt = nc.dram_tensor("wkt", [D, D], F32, kind="ExternalInput")
    xit = nc.dram_tensor("xit", [D, B], F32, kind="ExternalInput")
    out = nc.dram_tensor("out", [B, D], F32, kind="ExternalOutput")

    def pair_st(t, u):
        # stationary fp8 pair view [P, 2, P] of a [P, D] tile, pair u
        return t[:, u * 2 * P:(u + 1) * 2 * P].rearrange("p (i m) -> p i m", i=2)

    with tile.TileContext(nc) as tc, ExitStack() as ctx:
        pers = ctx.enter_context(tc.tile_pool(name="pers", bufs=1))
        aith = [pers.tile([P, B], F32R, name=f"aith{k}") for k in range(KT)]
        h8a = pers.tile([P, KT, B], F8, name="h8a")
        l8a = pers.tile([P, KT, B], F8, name="l8a")
        gm = pers.tile([P, B], F32, name="gm")
        mxbc = pers.tile([P, B], F32, name="mxbc")
        ident = pers.tile([P, P], F32, name="ident")
        make_identity(nc, ident[:])
        nc.gpsimd.memset(gm[:], -1e30)

        # ---- phase 0: T1 = Wqs^T X_i^T ; A^T = Wk T1 (per B-half) ----
        with ExitStack() as p0:
            t1p = p0.enter_context(tc.tile_pool(name="t1p", bufs=1))
            wp = p0.enter_context(tc.tile_pool(name="wp", bufs=2))
            xip = p0.enter_context(tc.tile_pool(name="xip", bufs=1))
            auxp = p0.enter_context(tc.tile_pool(name="auxp", bufs=2))
            ps0 = p0.enter_context(tc.tile_pool(name="ps0", bufs=4, space="PSUM"))
            HB = 512
            hT1 = t1p.tile([P, KT, HB], F32R, name="hT1")
            lT1 = t1p.tile([P, KT, HB], F32R, name="lT1")
            for (lo, hi) in _chunks(B):
                # X_i^T half-split: hi/lo f32r, pipelined per g-tile so the
                # first step1 matmuls start as soon as g=0 lands
                xi_f = xip.tile([P, KT, HB], F32, name=f"xif{lo}", tag="xif")
                hxi = xip.tile([P, KT, HB], F32R, name=f"hxi{lo}", tag="hxi")
                lxi = xip.tile([P, KT, HB], F32R, name=f"lxi{lo}", tag="lxi")
                for g in range(KT):
                    nc.sync.dma_start(xi_f[:, g, :], xit.ap()[g * P:(g + 1) * P, lo:hi])
                    nc.scalar.copy(hxi[:, g, :], xi_f[:, g, :])
                    nc.vector.tensor_sub(xi_f[:, g, :], xi_f[:, g, :], hxi[:, g, :].bitcast(F32))
                    nc.vector.tensor_copy(lxi[:, g, :], xi_f[:, g, :])

                # step1: T1[r-tile, half] = sum_g Wqs[g,:][:, r]^T X^T[g, half]
                for r in range(KT):
                    wq_f = wp.tile([P, KT, P], F32, name=f"wqf{lo}_{r}", tag="wqf")
                    for g in range(KT):
                        nc.sync.dma_start(wq_f[:, g, :], wqs.ap()[g * P:(g + 1) * P, r * P:(r + 1) * P])
                    hwq = wp.tile([P, KT, P], F32R, name=f"hwq{lo}_{r}", tag="hwq")
                    nc.scalar.copy(hwq[:], wq_f[:])
                    nc.vector.tensor_sub(wq_f[:], wq_f[:], hwq[:].bitcast(F32))
                    lwq = wp.tile([P, KT, P], F32R, name=f"lwq{lo}_{r}", tag="lwq")
                    nc.vector.tensor_copy(lwq[:], wq_f[:])
                    t1ps = ps0.tile([P, HB], F32, name=f"t1ps{lo}_{r}", tag="pm")
                    n = 3 * KT
                    i = 0
                    for g in range(KT):
                        for (la, rb) in ((hwq, hxi), (hwq, lxi), (lwq, hxi)):
                            nc.tensor.matmul(t1ps[:], la[:, g, :], rb[:, g, :],
                                             start=(i == 0), stop=(i == n - 1))
                            i += 1
                    nc.vector.tensor_copy(hT1[:, r, :], t1ps[:])
                    nc.vector.tensor_sub(t1ps[:], t1ps[:], hT1[:, r, :].bitcast(F32))
                    nc.vector.tensor_copy(lT1[:, r, :], t1ps[:])

                # step2: A^T[d-tile, half] = sum_r Wk[:, r][d, :] T1[r, half]
                # (reuses the step1 weight-split slots: same tags/shapes)
                for d in range(KT):
                    wk_f = wp.tile([P, KT, P], F32, name=f"wkf{lo}_{d}", tag="wqf")
                    for r in range(KT):
                        nc.sync.dma_start(wk_f[:, r, :], wkt.ap()[r * P:(r + 1) * P, d * P:(d + 1) * P])
                    hwk = wp.tile([P, KT, P], F32R, name=f"hwk{lo}_{d}", tag="hwq")
                    nc.scalar.copy(hwk[:], wk_f[:])
                    nc.vector.tensor_sub(wk_f[:], wk_f[:], hwk[:].bitcast(F32))
                    lwk = wp.tile([P, KT, P], F32R, name=f"lwk{lo}_{d}", tag="lwq")
                    nc.vector.tensor_copy(lwk[:], wk_f[:])
                    pa = ps0.tile([P, HB], F32, name=f"pa{lo}_{d}", tag="pm")
                    n = 3 * KT
                    i = 0
                    for r in range(KT):
                        for (la, rb) in ((hwk, hT1), (hwk, lT1), (lwk, hT1)):
                            nc.tensor.matmul(pa[:], la[:, r, :], rb[:, r, :],
                                             start=(i == 0), stop=(i == n - 1))
                            i += 1
                    nc.vector.tensor_copy(aith[d][:, lo:hi], pa[:])
                    nc.scalar.mul(h8a[:, d, lo:hi], aith[d][:, lo:hi].bitcast(F32), S_HA)
                    al_f = auxp.tile([P, HB], F32, name=f"alf{lo}_{d}", tag="alf")
                    nc.vector.tensor_sub(al_f[:], pa[:], aith[d][:, lo:hi].bitcast(F32))
                    nc.vector.tensor_scalar_mul(l8a[:, d, lo:hi], al_f[:], S_LA)

        # ---- flash over key super-blocks ----
        accp = ctx.enter_context(tc.tile_pool(name="accp", bufs=1))
        acc = [accp.tile([P, XAW], F32, name=f"acc{t}") for t in range(MT)]
        for t in range(MT):
            nc.gpsimd.memset(acc[t][:], 0.0)
        sp = ctx.enter_context(tc.tile_pool(name="sp", bufs=2 * sbn + 2))
        erp = ctx.enter_context(tc.tile_pool(name="erp", bufs=2 * sbn))
        xap = ctx.enter_context(tc.tile_pool(name="xap", bufs=3))
        xarp = ctx.enter_context(tc.tile_pool(name="xarp", bufs=2 * sbn))
        xtp = xap
        xthp = ctx.enter_context(tc.tile_pool(name="xthp", bufs=3))
        stat = ctx.enter_context(tc.tile_pool(name="stat", bufs=2))
        ps_s = ctx.enter_context(tc.tile_pool(name="ps_s", bufs=2, space="PSUM"))
        ps_o = ctx.enter_context(tc.tile_pool(name="ps_o", bufs=2, space="PSUM"))
        ps_t = ps_s

        # Software pipeline, one-super-block lag, tuned so the PE queue is
        # [S(s) | out(s-1) | transposes(s) | S(s+1) | ...] with no waits:
        # out(s-1)'s operands (er/xar/corr) were finished during S(s)'s burst,
        # and the gm column-maxes feeding transposes(s) finish during
        # out(s-1). The exp chain for s runs on DVE/ScalarE under S(s+1).
        # E and X_aug are bf16 for the out matmul (same 1 cyc/row as f32r,
        # half the SBUF; E's 2^-9 rounding cancels through the ones-column
        # denominator, X_aug's is ~2e-3 of |x| — both far under budget).
        # X_aug ships from the host already in bf16, so no on-device cast.
        def prep_block(s):
            js = list(range(s * sbn, (s + 1) * sbn))
            xsplit = []
            for j in js:
                xt_t = xtp.tile([P, XAW], F32, name=f"xt{j}", tag="stg")
                nc.sync.dma_start(xt_t[:, :D], xtj.ap()[j])
                xth = xthp.tile([P, D], F32R, name=f"xth{j}", tag="xth")
                nc.scalar.copy(xth[:], xt_t[:, :D])
                nc.vector.tensor_sub(xt_t[:, :D], xt_t[:, :D], xth[:].bitcast(F32))
                l8x = xthp.tile([P, D], F8, name=f"l8x{j}", tag="l8x")
                nc.vector.tensor_scalar_mul(l8x[:], xt_t[:, :D], S_LX)
                h8x = xthp.tile([P, D], F8, name=f"h8x{j}", tag="h8x")
                nc.scalar.mul(h8x[:], xth[:].bitcast(F32), S_HX)
                xsplit.append((xth, l8x, h8x))
            return xsplit

        def s_burst(s, xsplit):
            ssb = []
            for idx, j in enumerate(range(s * sbn, (s + 1) * sbn)):
                xth, l8x, h8x = xsplit[idx]
                s_t = sp.tile([P, B], F32, name=f"s{j}", tag="s")
                pss = [ps_s.tile([P, 512], F32, name=f"pss{j}_{c}", tag="pss")
                       for c in range(2)]
                # f32r hh pass, both chunks back-to-back (same PE mode)
                for c, (lo, hi) in enumerate(_chunks(B)):
                    for k in range(KT):
                        nc.tensor.matmul(pss[c][:], xth[:, k * P:(k + 1) * P],
                                         aith[k][:, lo:hi], start=(k == 0), stop=(k == KT - 1))
                # fp8 DoubleRow cross passes: each accumulates onto the
                # closed f32r group via has_written (start=False); stop is
                # sim-only bookkeeping so every DR matmul closes itself.
                # Chunk 0 drains (copy + running max) while chunk 1's fp8
                # matmuls stream, so the stats transposes can start the
                # moment the burst ends.
                for c, (lo, hi) in enumerate(_chunks(B)):
                    for u in range(NPAIR):
                        nc.tensor.matmul(pss[c][:], pair_st(l8x, u),
                                         h8a[:, 2 * u:2 * u + 2, lo:hi],
                                         start=False, stop=True, perf_mode=DR,
                                         skip_group_check=True)
                    for u in range(NPAIR):
                        nc.tensor.matmul(pss[c][:], pair_st(h8x, u),
                                         l8a[:, 2 * u:2 * u + 2, lo:hi],
                                         start=False, stop=True, perf_mode=DR,
                                         skip_group_check=True)
                    nc.scalar.copy(s_t[:, lo:hi], pss[c][:])
                    nc.vector.tensor_max(gm[:, lo:hi], gm[:, lo:hi], pss[c][:])
                ssb.append(s_t)
            return ssb

        def stats_block(s, omx):
            # per-query-column running max (transpose-reduce gm chunks)
            nmx = stat.tile([P, MT], F32, name=f"nmx{s}", tag="nmx")
            corr = stat.tile([P, MT], F32, name=f"corr{s}", tag="corr")
            for c in range(MT):
                pt = ps_t.tile([P, P], F32, name=f"pt{s}_{c}", tag="pss")
                nc.tensor.transpose(pt[:], gm[:, c * P:(c + 1) * P], ident[:])
                nc.vector.reduce_max(nmx[:, c:c + 1], pt[:], axis=AXX)
            if omx is None:
                nc.vector.memset(corr[:], 0.0)
            else:
                dmx = stat.tile([P, MT], F32, name=f"dmx{s}", tag="dmx")
                nc.vector.tensor_sub(dmx[:], omx[:], nmx[:])
                nc.scalar.activation(corr[:], dmx[:], EXP)

            # broadcast nmx (query-major) -> mxbc [P, B] (key-major free)
            ptb = ps_t.tile([P, P], F32, name=f"ptb{s}", tag="pss")
            nc.tensor.transpose(ptb[:MT, :], nmx[:], ident[:])
            mtmp = stat.tile([MT, P], F32, name=f"mtmp{s}", tag="mtmp")
            nc.scalar.copy(mtmp[:], ptb[:MT, :])
            # issue the tiny mrow DMA from the scalar queue so it doesn't
            # sit behind the bulk xtj/xa loads on the sync queue
            mrow = stat.tile([1, B], F32, name=f"mrow{s}", tag="mrow", bufs=1)
            nc.scalar.dma_start(mrow[:].rearrange("a (b c) -> a b c", b=MT), mtmp[:])
            nc.gpsimd.partition_broadcast(mxbc[:], mrow[:])
            return nmx, corr

        def exp_block(s, ssb):
            # E = exp(S - max), exp writes bf16 er (out dtype converts).
            # Chunked [P, 512] with chunk 0 of every tile first: the out
            # burst consumes er columns t*128.. in t order, so all its
            # early stationaries come from chunk 0 — this halves the time
            # from max-broadcast to out-burst start.
            ers = [erp.tile([P, B], BF16, name=f"er{s}_{idx}", tag="er")
                   for idx in range(len(ssb))]
            for (lo, hi) in _chunks(B):
                for idx, s_t in enumerate(ssb):
                    nc.vector.tensor_sub(s_t[:, lo:hi], s_t[:, lo:hi], mxbc[:, lo:hi])
                    nc.scalar.activation(ers[idx][:, lo:hi], s_t[:, lo:hi], EXP)
            return ers

        def xar_block(s):
            xar = []
            for j in range(s * sbn, (s + 1) * sbn):
                xa_t = xarp.tile([P, XAW], BF16, name=f"xa{j}", tag="xar")
                nc.sync.dma_start(xa_t[:], xa.ap()[j * P:(j + 1) * P, :])
                xar.append(xa_t)
            return xar

        def out_block(s, ers, xar, corr):
            # out accumulation: acc = acc*corr + E^T @ X_aug (f32r burst);
            # 3 matmul streams into bank-aligned slices of one PSUM tile,
            # then a single fused rescale-accumulate per query tile
            for t in range(MT):
                po = ps_o.tile([P, XAW], F32, name=f"po{s}_{t}", tag="po")
                # idx outer so the 3 column chunks reuse one stationary
                # operand back-to-back (LDWEIGHTS locality); each chunk's
                # PSUM accumulation group still spans idx 0..sbn-1
                for idx in range(sbn):
                    er = ers[idx][:]
                    for (lo, hi) in _chunks(XAW):
                        nc.tensor.matmul(po[:, lo:hi], er[:, t * P:(t + 1) * P],
                                         xar[idx][:, lo:hi], start=(idx == 0), stop=(idx == sbn - 1))
                nc.vector.scalar_tensor_tensor(acc[t][:], acc[t][:],
                                               corr[:, t:t + 1], po[:],
                                               op0=ALU.mult, op1=ALU.add)

        omx = None
        prev = None  # (ers, xar, corr) of block s-1
        xsplit = prep_block(0)
        for s in range(NSB):
            ssb = s_burst(s, xsplit)
            if s + 1 < NSB:
                xsplit = prep_block(s + 1)
            # stats(s) on PE right after the burst (its gm maxes already
            # drained), so the max-broadcast/exp chain overlaps out(s-1)
            nmx, corr = stats_block(s, omx)
            omx = nmx
            if prev is not None:
                out_block(s - 1, *prev)
            ers = exp_block(s, ssb)
            xar = xar_block(s)
            prev = (ers, xar, corr)
        out_block(NSB - 1, *prev)

        # ---- finalize: divide by the ones-column sums, write out ----
        if True:
            for t in range(MT):
                rc = stat.tile([P, 1], F32, name=f"rc{t}", tag="rc")
                nc.vector.reciprocal(rc[:], acc[t][:, D:D + 1])
                nc.vector.tensor_scalar_mul(acc[t][:, 0:D], acc[t][:, 0:D], rc[:])
                nc.sync.dma_start(out.ap()[t * P:(t + 1) * P, :], acc[t][:, 0:D])

    nc.compile()
    return nc


def prep_inputs(X, Wq, Wk, S, D, n_cores, aug=AUG):
    B = S // n_cores
    NT = S // P
    KT = D // P
    X = np.ascontiguousarray(X, np.float32)
    scale = np.float32(1.0 / np.sqrt(D))
    xtj = np.ascontiguousarray(
        X.reshape(NT, P, KT, P).transpose(0, 3, 2, 1).reshape(NT, P, D))
    import ml_dtypes
    xa = np.zeros((S, D + aug), ml_dtypes.bfloat16)
    xa[:, :D] = X.astype(ml_dtypes.bfloat16)
    xa[:, D] = 1.0
    wqs = np.ascontiguousarray(np.asarray(Wq, np.float32) * scale)
    wkt = np.ascontiguousarray(np.asarray(Wk, np.float32).T)
    xt = X.T
    in_maps = []
    for i in range(n_cores):
        in_maps.append({
            "xtj": xtj, "xa": xa, "wqs": wqs, "wkt": wkt,
            "xit": np.ascontiguousarray(xt[:, i * B:(i + 1) * B]),
        })
    return in_maps


_CACHE = {}


def _get_kernel(S, D, B, sbn):
    key = (S, D, B, sbn)
    if key not in _CACHE:
        _CACHE[key] = build_core_kernel(S, D, B, sbn=sbn)
    return _CACHE[key]


def kernel(inputs, weight_query, weight_key):
    S, D = inputs.shape
    assert (S, D) == (SEQ, DIM)
    B = S // NCORES
    nc = _get_kernel(S, D, B, SBN)
    in_maps = prep_inputs(inputs, weight_query, weight_key, S, D, NCORES)
    res = run_bass_kernel_spmd(nc, in_maps, core_ids=list(range(NCORES)))
    return np.concatenate([res.results[i]["out"] for i in range(NCORES)], axis=0)


if __name__ == "__main__":
    rng = np.random.default_rng(0)
    X = rng.standard_normal((SEQ, DIM), dtype=np.float32)
    Wq = rng.standard_normal((DIM, DIM), dtype=np.float32)
    Wk = rng.standard_normal((DIM, DIM), dtype=np.float32)
    out = kernel(X, Wq, Wk)
    print(out.shape, out.dtype)
